# revision 22
# baseline (speedup 1.0000x reference)
"""Trainium2 Bass kernel for nn_DiscreteQKTRBlock (sparse 3x3x3 neighborhood
attention with a discrete codebook).

Strategy (data-parallel over points, 8 cores), v2 "edge-expanded halo":

The discrete-codebook STE path collapses algebraically:
    s[k,i]  = dq[i] . dq[nbr[k,i]] = ||cb||^2 * choice[i] * choice[nbr[k,i]]
so per-offset scores reduce to scalar products of `choice'` = sqrt(cb2)*choice.

Host-side, neighbor indices are fully known, so we pre-expand a "halo" copy of
x per edge slot (xeT, feature-major fp16).  The device then needs NO random
DRAM gathers for x-dependent data:

  A) per consumer tile: q^T = sum_k Wq_k.T @ xe_k  (PSUM accumulation),
     choice' per own point -> strip
  B) AllGather strip (50KB/core); build a per-partition-replicated SBUF table
     of all 100K choice' values (fp16, two 98KB slabs) and resolve per-edge
     neighbor choice via gpsimd ap_gather + diagonal-mask extraction -> ce
  C) per consumer tile: scores = strip*ce + bias, masked softmax; per-slot
     v^T = relu(Wv.T @ xe_k + beta), PE-transpose, weighted DVE accumulation;
     pos is aggregated as sum_k w_k*coords4 and folded through
     (Wpos_exp @ W_out) into the output matmul; relu + residual.

All weight-affine folds are host-side weight-space transforms only.
"""
import sys
sys.path.insert(0, "/opt/trn_rl_repo")
import numpy as np
import ml_dtypes

from concourse import bass, bacc, mybir
import concourse.tile as tile
from concourse.bass_utils import run_bass_kernel_spmd
from concourse.masks import make_identity

F32 = mybir.dt.float32
FP16 = mybir.dt.float16
I16 = mybir.dt.int16
I32 = mybir.dt.int32

N = 100000
P = 128
VEC = 16
K = 27
NEG = -1e9
NCORE = 8
NSH = 12544                 # points per core (98 tiles of 128)
TO = NSH // P               # 98 own tiles
NTOT = NCORE * NSH          # 100352 global (padded) points
Z = N                       # new-id of the guaranteed all-zero pad row
COLS = NCORE * TO           # 784 columns in the wrapped choice layout
HALFV = NTOT // 2           # 50176 choice values per table slab
ENT = HALFV // 2 + 1        # 25089 entries per slab (d=2, incl. zero entry)

_CACHE = {}


def _build_nc(kts, use_bch, use_vb):
    SUMK = sum(kts)
    so = [int(v) for v in np.concatenate([[0], np.cumsum(kts)])]  # slot offsets
    H1 = TO // 2

    nc = bacc.Bacc(num_devices=NCORE, dynamic_dma_scratch_size=16384)

    # ---------------- inputs ----------------
    xeA = nc.declare_dram_parameter("xeA", [P, TO * K * P], FP16, isOutput=False)
    xeT = nc.declare_dram_parameter("xeT", [P, SUMK * P], FP16, isOutput=False)
    aux = nc.declare_dram_parameter("aux", [P, SUMK * 5], F32, isOutput=False)
    pki = nc.declare_dram_parameter("pki", [P, SUMK * 2], I16, isOutput=False)
    pkc = nc.declare_dram_parameter("pkc", [P, SUMK], FP16, isOutput=False)
    xT_own = nc.declare_dram_parameter("xT_own", [P, NSH], F32, isOutput=False)
    w_q = nc.declare_dram_parameter("w_q", [P, K * VEC], FP16, isOutput=False)
    wcc_in = nc.declare_dram_parameter("wcc", [VEC, P], F32, isOutput=False)
    bch_in = nc.declare_dram_parameter("bch", [1, P], F32, isOutput=False)
    wv_in = nc.declare_dram_parameter("wv", [P, P], FP16, isOutput=False)
    wo_in = nc.declare_dram_parameter("wo", [P, P], FP16, isOutput=False)
    wpw_in = nc.declare_dram_parameter("wpw", [4, P], FP16, isOutput=False)
    if use_vb:
        vbr_in = nc.declare_dram_parameter("vbr", [1, P], FP16, isOutput=False)
    qg_in = nc.declare_dram_parameter("qg", [VEC, 1], F32, isOutput=False)
    qb_in = nc.declare_dram_parameter("qb", [VEC, 1], F32, isOutput=False)
    vbeta_in = nc.declare_dram_parameter("vbeta", [P, 1], F32, isOutput=False)
    obeta_in = nc.declare_dram_parameter("obeta", [P, 1], F32, isOutput=False)
    rmio_in = nc.declare_dram_parameter("rmio", [P, 32], FP16, isOutput=False)

    outT = nc.declare_dram_parameter("outT", [P, NSH], F32, isOutput=True)

    AF = mybir.ActivationFunctionType
    ALU = mybir.AluOpType

    with tile.TileContext(nc) as tc:
        with tc.tile_pool(name="persist", bufs=1) as pp, \
             tc.tile_pool(name="dram", bufs=1, space="DRAM") as dpool:
            strip = pp.tile([P, TO], F32)
            qg_sb = pp.tile([VEC, 1], F32)
            nc.sync.dma_start(out=qg_sb[:], in_=qg_in[:, :])
            qb_sb = pp.tile([VEC, 1], F32)
            nc.sync.dma_start(out=qb_sb[:], in_=qb_in[:, :])
            vbeta_sb = pp.tile([P, 1], F32)
            nc.sync.dma_start(out=vbeta_sb[:], in_=vbeta_in[:, :])
            obeta_sb = pp.tile([P, 1], F32)
            nc.sync.dma_start(out=obeta_sb[:], in_=obeta_in[:, :])
            zero_col = pp.tile([P, 1], F32)
            nc.vector.memset(zero_col[:], 0.0)

            c16d = dpool.tile([P, COLS], FP16)
            ced = dpool.tile([P, SUMK], FP16)
            cc_in1 = dpool.tile([P, H1], F32)
            cc_out1 = dpool.tile([NCORE, P, H1], F32, addr_space="Shared")
            cc_in2 = dpool.tile([P, TO - H1], F32)
            cc_out2 = dpool.tile([NCORE, P, TO - H1], F32, addr_space="Shared")

            # ================= scope 1: phase A + allgather =================
            with tc.tile_pool(name="a_const", bufs=1) as acp, \
                 tc.tile_pool(name="a_xe", bufs=3) as axp, \
                 tc.tile_pool(name="a_w", bufs=3) as awp, \
                 tc.tile_pool(name="a_ps", bufs=2, space="PSUM") as apsp, \
                 tc.tile_pool(name="a_ps2", bufs=2, space="PSUM") as apsp2:
                wq_sb = acp.tile([P, K * VEC], FP16)
                nc.sync.dma_start(out=wq_sb[:], in_=w_q[:, :])
                wcc_sb = acp.tile([VEC, P], F32)
                nc.sync.dma_start(out=wcc_sb[:], in_=wcc_in[:, :])
                if use_bch:
                    bch_sb = acp.tile([1, P], F32)
                    nc.sync.dma_start(out=bch_sb[:], in_=bch_in[:, :])
                    ones1 = acp.tile([1, P], F32)
                    nc.vector.memset(ones1[:], 1.0)

                with nc.named_scope("phaseA"):
                    for tg in range(0, TO, 4):
                        nt = min(4, TO - tg)
                        xe4 = axp.tile([P, 4 * K * P], FP16, tag="xe")
                        nc.sync.dma_start(
                            out=xe4[:, 0:nt * K * P],
                            in_=xeA[:, tg * K * P:(tg + nt) * K * P])
                        q4 = apsp.tile([VEC, 4 * P], F32, tag="q",
                                       padded_shape=[P, 4 * P])
                        for k in range(K):
                            rhs = bass.AP(xe4.tensor, xe4[:].offset + k * P,
                                          [xe4[:].ap[0], (K * P, nt), (1, P)])
                            nc.tensor.matmul(
                                out=q4[:, 0:nt * P],
                                lhsT=wq_sb[:, k * VEC:(k + 1) * VEC],
                                rhs=rhs, start=(k == 0), stop=(k == K - 1))
                        qf = awp.tile([VEC, 4 * P], F32, tag="qf")
                        nc.scalar.activation(
                            out=qf[:, 0:nt * P], in_=q4[:, 0:nt * P],
                            func=AF.Relu, bias=qb_sb[:, 0:1],
                            scale=qg_sb[:, 0:1])
                        for j in range(nt):
                            t = tg + j
                            t_ps = apsp2.tile([P, P], F32, tag="t")
                            if use_bch:
                                nc.tensor.matmul(
                                    out=t_ps[:], lhsT=qf[:, j * P:(j + 1) * P],
                                    rhs=wcc_sb[:], start=True, stop=False)
                                nc.tensor.matmul(
                                    out=t_ps[:], lhsT=ones1[:], rhs=bch_sb[:],
                                    start=False, stop=True)
                            else:
                                nc.tensor.matmul(
                                    out=t_ps[:], lhsT=qf[:, j * P:(j + 1) * P],
                                    rhs=wcc_sb[:], start=True, stop=True)
                            scratch = awp.tile([P, P], FP16, tag="scr")
                            nc.scalar.activation(
                                out=scratch[:], in_=t_ps[:], func=AF.Relu,
                                accum_out=strip[:, t:t + 1])

                with nc.named_scope("gather_choice"):
                    nc.sync.dma_start(out=cc_in1[:], in_=strip[:, 0:H1])
                    nc.gpsimd.collective_compute(
                        "AllGather", ALU.bypass,
                        replica_groups=[list(range(NCORE))],
                        ins=[cc_in1.opt()], outs=[cc_out1.opt()])
                    nc.sync.dma_start(out=cc_in2[:], in_=strip[:, H1:TO])
                    nc.gpsimd.collective_compute(
                        "AllGather", ALU.bypass,
                        replica_groups=[list(range(NCORE))],
                        ins=[cc_in2.opt()], outs=[cc_out2.opt()])

            # ================= scope 2a: choice table to DRAM ===============
            with tc.tile_pool(name="b_ch", bufs=1) as bchp:
                with nc.named_scope("chprep"):
                    ch32 = bchp.tile([P, COLS], F32)
                    ca_rt = ch32[:, 0:COLS].rearrange("p (r t) -> p r t", r=NCORE)
                    nc.sync.dma_start(
                        out=ca_rt[:, :, 0:H1],
                        in_=cc_out1[:, :, :].rearrange("r p t -> p r t"))
                    nc.sync.dma_start(
                        out=ca_rt[:, :, H1:TO],
                        in_=cc_out2[:, :, :].rearrange("r p t -> p r t"))
                    ch16 = bchp.tile([P, COLS], FP16)
                    nc.vector.tensor_copy(out=ch16[:], in_=ch32[:])
                    nc.sync.dma_start(out=c16d[:, :], in_=ch16[:])

            # ================= scope 2b: per-edge choice (ce) ===============
            with tc.tile_pool(name="c_fix", bufs=1) as cfp, \
                 tc.tile_pool(name="c_tab", bufs=1) as ctp, \
                 tc.tile_pool(name="c_pk", bufs=5) as cpkp, \
                 tc.tile_pool(name="c_raw", bufs=5) as crawp, \
                 tc.tile_pool(name="c_w", bufs=4) as cwp:
                rm_sb = cfp.tile([P, 32], FP16)
                nc.sync.dma_start(out=rm_sb[:], in_=rmio_in[:, :])
                celo = cfp.tile([P, SUMK], F32)


                with nc.named_scope("cepass"):
                    for s in range(2):
                        tab = ctp.tile([P, 2 * ENT], FP16, tag="tab")
                        nc.vector.memset(tab[:, 0:2], 0.0)
                        src = bass.AP(c16d.tensor, s * HALFV,
                                      [(0, P), (1, HALFV)])
                        nc.sync.dma_start(out=tab[:, 2:2 + HALFV], in_=src)
                        for t in range(TO):
                            KT = kts[t]
                            pki_t = cpkp.tile([P, KT], I16, tag="pki")
                            nc.sync.dma_start(
                                out=pki_t[:],
                                in_=pki[:, so[t] * 2 + s * KT:
                                        so[t] * 2 + (s + 1) * KT])
                            code_t = cpkp.tile([P, KT], FP16, tag="pkc")
                            nc.scalar.dma_start(
                                out=code_t[:], in_=pkc[:, so[t]:so[t] + KT])
                            raw = crawp.tile([P, 16 * KT * 2], FP16, tag="raw")
                            nc.gpsimd.ap_gather(
                                out_ap=raw[:].rearrange("p (n d) -> p n d", d=2),
                                in_ap=tab[:].rearrange("p (n d) -> p n d", d=2),
                                idxs_ap=pki_t[:, 0:KT],
                                channels=P, num_elems=ENT, d=2,
                                num_idxs=16 * KT)
                            mask = cwp.tile([P, KT * 32], FP16, tag="mk")
                            code_bc = bass.AP(code_t.tensor, code_t[:].offset,
                                              [code_t[:].ap[0], (1, KT),
                                               (0, 32)])
                            rm_bc = bass.AP(rm_sb.tensor, rm_sb[:].offset,
                                            [rm_sb[:].ap[0], (0, KT), (1, 32)])
                            nc.vector.tensor_tensor(
                                out=mask[:].rearrange("p (a b) -> p a b", b=32),
                                in0=code_bc, in1=rm_bc, op=ALU.is_equal)
                            prod = cwp.tile([P, KT * 32], FP16, tag="pr")
                            nc.vector.tensor_tensor(
                                out=prod[:], in0=raw[:], in1=mask[:],
                                op=ALU.mult)
                            if s == 0:
                                nc.vector.tensor_reduce(
                                    out=celo[:, so[t]:so[t] + KT],
                                    in_=prod[:].rearrange(
                                        "p (a b) -> p a b", b=32),
                                    axis=mybir.AxisListType.X, op=ALU.add)
                            else:
                                cet = cwp.tile([P, KT], F32, tag="cet")
                                nc.vector.tensor_reduce(
                                    out=cet[:],
                                    in_=prod[:].rearrange(
                                        "p (a b) -> p a b", b=32),
                                    axis=mybir.AxisListType.X, op=ALU.add)
                                ce16 = cwp.tile([P, KT], FP16, tag="ce16")
                                nc.vector.tensor_tensor(
                                    out=ce16[:], in0=cet[:],
                                    in1=celo[:, so[t]:so[t] + KT], op=ALU.add)
                                nc.scalar.dma_start(
                                    out=ced[:, so[t]:so[t] + KT], in_=ce16[:])

            # ================= scope 3: phase C =============================
            with tc.tile_pool(name="d_const", bufs=1) as dcp, \
                 tc.tile_pool(name="d_xe", bufs=3) as dxp, \
                 tc.tile_pool(name="d_aux", bufs=4) as dauxp, \
                 tc.tile_pool(name="d_w", bufs=4) as dwp, \
                 tc.tile_pool(name="d_vps", bufs=3, space="PSUM") as dvps, \
                 tc.tile_pool(name="d_tps", bufs=2, space="PSUM") as dtps, \
                 tc.tile_pool(name="d_t1ps", bufs=1, space="PSUM") as dt1ps, \
                 tc.tile_pool(name="d_ops", bufs=1, space="PSUM") as dops:
                wv_sb = dcp.tile([P, P], FP16)
                nc.sync.dma_start(out=wv_sb[:], in_=wv_in[:, :])
                wo_sb = dcp.tile([P, P], FP16)
                nc.sync.dma_start(out=wo_sb[:], in_=wo_in[:, :])
                wpw_sb = dcp.tile([4, P], FP16)
                nc.sync.dma_start(out=wpw_sb[:], in_=wpw_in[:, :])
                ident16 = dcp.tile([P, P], FP16)
                make_identity(nc, ident16[:])
                aux_sb = dcp.tile([P, SUMK * 5], F32)
                nc.sync.dma_start(out=aux_sb[:], in_=aux[:, :])
                if use_vb:
                    vbr_sb = dcp.tile([1, P], FP16)
                    nc.sync.dma_start(out=vbr_sb[:], in_=vbr_in[:, :])
                    ones1f = dcp.tile([1, P], FP16)
                    nc.vector.memset(ones1f[:], 1.0)

                with nc.named_scope("phaseC"):
                    for t in range(TO):
                        KT = kts[t]
                        xe_t = dxp.tile([P, KT * P], FP16, tag="xe")
                        nc.sync.dma_start(
                            out=xe_t[:], in_=xeT[:, so[t] * P:(so[t] + KT) * P])
                        ce_t = dauxp.tile([P, KT], FP16, tag="ce")
                        nc.scalar.dma_start(
                            out=ce_t[:], in_=ced[:, so[t]:so[t] + KT])
                        xo_t = dauxp.tile([P, P], F32, tag="xo")
                        nc.sync.dma_start(
                            out=xo_t[:], in_=xT_own[:, t * P:(t + 1) * P])

                        # scores + masked softmax
                        s_t = dwp.tile([P, KT], F32, tag="s")
                        bias_view = bass.AP(aux_sb.tensor,
                                            aux_sb[:].offset + so[t] * 5 + 4,
                                            [aux_sb[:].ap[0], (5, KT)])
                        nc.vector.scalar_tensor_tensor(
                            out=s_t[:], in0=ce_t[:], scalar=strip[:, t:t + 1],
                            in1=bias_view, op0=ALU.mult, op1=ALU.add)
                        negmax = dwp.tile([P, 1], F32, tag="nm")
                        nc.vector.tensor_reduce(
                            out=negmax[:], in_=s_t[:], axis=mybir.AxisListType.X,
                            op=ALU.max, negate=True)
                        e_t = dwp.tile([P, KT], F32, tag="e")
                        esum = dwp.tile([P, 1], F32, tag="es")
                        nc.scalar.activation(
                            out=e_t[:], in_=s_t[:], func=AF.Exp,
                            bias=negmax[:, 0:1], scale=1.0,
                            accum_out=esum[:, 0:1])
                        rs = dwp.tile([P, 1], F32, tag="rsx")
                        nc.vector.reciprocal(out=rs[:], in_=esum[:])
                        w_t = dwp.tile([P, KT], F32, tag="w")
                        nc.vector.tensor_scalar_mul(out=w_t[:], in0=e_t[:],
                                                    scalar1=rs[:, 0:1])

                        # pos: aggregate coords4 with attn weights
                        c4_view = bass.AP(aux_sb.tensor,
                                          aux_sb[:].offset + so[t] * 5,
                                          [aux_sb[:].ap[0], (5, KT), (1, 4)])
                        w_bc = bass.AP(w_t.tensor, w_t[:].offset,
                                       [w_t[:].ap[0], (1, KT), (0, 4)])
                        tmp4 = dwp.tile([P, KT * 4], F32, tag="t4")
                        nc.vector.tensor_tensor(
                            out=tmp4[:].rearrange("p (a b) -> p a b", b=4),
                            in0=c4_view, in1=w_bc, op=ALU.mult)
                        ag4 = dwp.tile([P, 4], F32, tag="a4")
                        ag4_in = bass.AP(tmp4.tensor, tmp4[:].offset,
                                         [tmp4[:].ap[0], (1, 4), (4, KT)])
                        nc.vector.tensor_reduce(
                            out=ag4[:], in_=ag4_in, axis=mybir.AxisListType.X,
                            op=ALU.add)
                        ag416 = dwp.tile([P, 4], FP16, tag="a416")
                        nc.scalar.copy(out=ag416[:], in_=ag4[:])
                        a4T_ps = dt1ps.tile([4, P], FP16, tag="a4T",
                                            padded_shape=[P, P])
                        nc.tensor.transpose(out=a4T_ps[:], in_=ag416[:],
                                            identity=ident16[:])
                        a4T = dwp.tile([4, P], FP16, tag="a4Ts")
                        nc.scalar.copy(out=a4T[:], in_=a4T_ps[:])

                        # weighted aggregation of v (points on out partitions)
                        accA = dwp.tile([P, P], FP16, tag="accA")
                        accB = dwp.tile([P, P], FP16, tag="accB")
                        for k0 in range(0, KT, 4):
                            nk = min(4, KT - k0)
                            v4 = dvps.tile([P, 4 * P], F32, tag="v")
                            for j in range(nk):
                                if use_vb:
                                    nc.tensor.matmul(
                                        out=v4[:, j * P:(j + 1) * P],
                                        lhsT=xe_t[:, (k0 + j) * P:
                                                  (k0 + j + 1) * P],
                                        rhs=wv_sb[:], start=True, stop=False)
                                    nc.tensor.matmul(
                                        out=v4[:, j * P:(j + 1) * P],
                                        lhsT=ones1f[:], rhs=vbr_sb[:],
                                        start=False, stop=True)
                                else:
                                    nc.tensor.matmul(
                                        out=v4[:, j * P:(j + 1) * P],
                                        lhsT=xe_t[:, (k0 + j) * P:
                                                  (k0 + j + 1) * P],
                                        rhs=wv_sb[:], start=True, stop=True)
                            vT4 = dwp.tile([P, 4 * P], FP16, tag="vT")
                            if (k0 // 4) % 2 == 0:
                                nc.scalar.activation(
                                    out=vT4[:, 0:nk * P], in_=v4[:, 0:nk * P],
                                    func=AF.Relu)
                            else:
                                nc.vector.tensor_scalar_max(
                                    out=vT4[:, 0:nk * P], in0=v4[:, 0:nk * P],
                                    scalar1=0.0)
                            for j in range(nk):
                                k = k0 + j
                                sl = vT4[:, j * P:(j + 1) * P]
                                wk = w_t[:, k:k + 1]
                                if k == 0:
                                    nc.vector.tensor_scalar_mul(
                                        out=accA[:], in0=sl, scalar1=wk)
                                elif k == 1:
                                    nc.vector.tensor_scalar_mul(
                                        out=accB[:], in0=sl, scalar1=wk)
                                elif k % 2 == 0:
                                    nc.vector.scalar_tensor_tensor(
                                        out=accA[:], in0=sl, scalar=wk,
                                        op0=ALU.mult, in1=accA[:], op1=ALU.add)
                                else:
                                    nc.vector.scalar_tensor_tensor(
                                        out=accB[:], in0=sl, scalar=wk,
                                        op0=ALU.mult, in1=accB[:], op1=ALU.add)
                        acc = dwp.tile([P, P], FP16, tag="acc")
                        if KT == 1:
                            nc.vector.tensor_copy(out=acc[:], in_=accA[:])
                        else:
                            nc.vector.tensor_tensor(
                                out=acc[:], in0=accA[:], in1=accB[:],
                                op=ALU.add)

                        accT_ps = dt1ps.tile([P, P], FP16, tag="accT")
                        nc.tensor.transpose(out=accT_ps[:], in_=acc[:],
                                            identity=ident16[:])
                        accT = dwp.tile([P, P], FP16, tag="accTs")
                        nc.scalar.copy(out=accT[:], in_=accT_ps[:])
                        o_ps = dops.tile([P, P], F32, tag="o")
                        nc.tensor.matmul(out=o_ps[:], lhsT=wo_sb[:], rhs=accT[:],
                                         start=True, stop=False)
                        nc.tensor.matmul(out=o_ps[:], lhsT=wpw_sb[:], rhs=a4T[:],
                                         start=False, stop=True)
                        oT = dwp.tile([P, P], F32, tag="oT")
                        nc.scalar.activation(
                            out=oT[:], in_=o_ps[:], func=AF.Relu,
                            bias=obeta_sb[:, 0:1])
                        res = dwp.tile([P, P], F32, tag="res")
                        nc.vector.tensor_tensor(out=res[:], in0=oT[:],
                                                in1=xo_t[:], op=ALU.add)
                        nc.sync.dma_start(out=outT[:, t * P:(t + 1) * P],
                                          in_=res[:])

    nc.finalize()
    return nc


def _prep(inputs):
    x = np.asarray(inputs["x"], np.float32)
    coords = np.asarray(inputs["coords"], np.float32)
    W_q = np.asarray(inputs["W_q"], np.float32)
    q_gamma = np.asarray(inputs["q_gamma"], np.float32)
    q_beta = np.asarray(inputs["q_beta"], np.float32)
    W_v = np.asarray(inputs["W_v"], np.float32)
    v_gamma = np.asarray(inputs["v_gamma"], np.float32)
    v_beta = np.asarray(inputs["v_beta"], np.float32)
    codebook = np.asarray(inputs["codebook"], np.float32)
    W_choice = np.asarray(inputs["W_choice"], np.float32)
    b_choice = np.asarray(inputs["b_choice"], np.float32)
    W_pos = np.asarray(inputs["W_pos"], np.float32)
    b_pos = np.asarray(inputs["b_pos"], np.float32)
    W_out = np.asarray(inputs["W_out"], np.float32)
    out_gamma = np.asarray(inputs["out_gamma"], np.float32)
    out_beta = np.asarray(inputs["out_beta"], np.float32)
    nbr_idx = np.asarray(inputs["nbr_idx"], np.int32)
    nbr_mask = np.asarray(inputs["nbr_mask"], np.int32)

    n = x.shape[0]
    assert n == N

    # ---- valid-degree sort (per core shard) -> global relabeling ----
    mask_pad = np.zeros((K, NTOT), bool)
    mask_pad[:, :n] = nbr_mask > 0
    deg = mask_pad.sum(0)
    orders = []
    degs_sorted = np.empty((NCORE, NSH), np.int64)
    for r in range(NCORE):
        sl = slice(r * NSH, (r + 1) * NSH)
        o = np.argsort(-deg[sl], kind="stable")
        orders.append(o)
        degs_sorted[r] = deg[sl][o]
    kts = tuple(int(max(1, degs_sorted[:, t * P:(t + 1) * P].max()))
                for t in range(TO))
    SUMK = sum(kts)
    perm_full = np.concatenate([r * NSH + orders[r] for r in range(NCORE)])
    inv = np.empty(NTOT, np.int64)
    inv[perm_full] = np.arange(NTOT)

    # ---- permuted global tables (new-id order) ----
    xp = np.zeros((NTOT, P), np.float32)
    xp[:n] = x
    xp2 = xp[perm_full]
    x16g = xp2.astype(np.float16)
    cp = np.zeros((NTOT, 3), np.float32)
    cp[:n] = coords
    c4g = np.ones((NTOT, 4), np.float32)
    c4g[:, :3] = cp[perm_full]

    # ---- weight folds ----
    cb2 = float(np.dot(codebook, codebook))
    scb = np.sqrt(cb2).astype(np.float32)
    wcp = codebook[:, None] * W_choice
    wcc = scb * wcp.reshape(VEC, P // VEC, P).sum(1)
    bch = (scb * b_choice)[None, :]
    use_bch = bool(np.any(b_choice != 0))
    wq_flat = np.ascontiguousarray(
        W_q.transpose(1, 0, 2).reshape(P, K * VEC)).astype(np.float16)
    wv16 = (W_v * v_gamma[None, :]).astype(np.float16)
    use_vb = bool(np.any(v_beta != 0))
    wo = W_out * out_gamma[None, :]
    wo16 = wo.astype(np.float16)
    woB = wo.reshape(VEC, P // VEC, P).sum(1)          # [16, 128]
    wpos4 = np.concatenate([W_pos, b_pos[None, :]], axis=0)  # [4, 16]
    wpw16 = (wpos4 @ woB).astype(np.float16)           # [4, 128]
    rmio = np.tile(np.arange(32, dtype=np.float16)[None, :], (P, 1))

    # ---- per-slot neighbor ids (new ids, valid-first compaction) ----
    idx_new = np.full((K, NTOT), Z, np.int32)
    idx_new[:, :n] = np.where(nbr_mask > 0, inv[nbr_idx], Z).astype(np.int32)
    bias_pad = np.full((K, NTOT), np.float32(NEG), np.float32)
    bias_pad[:, :n] = np.where(nbr_mask > 0, 0.0, NEG).astype(np.float32)
    idx_km = idx_new[:, perm_full]          # k-major (original offsets)
    korder = np.argsort(~mask_pad, axis=0, kind="stable")   # valid ks first
    idx_new = np.take_along_axis(idx_new, korder, axis=0)
    bias_pad = np.take_along_axis(bias_pad, korder, axis=0)
    # permute slot-grid columns to sorted point order
    idx_new = idx_new[:, perm_full]
    bias_pad = bias_pad[:, perm_full]

    shared = dict(w_q=wq_flat, wcc=wcc, bch=bch, wv=wv16, wo=wo16,
                  wpw=wpw16, qg=q_gamma[:, None], qb=q_beta[:, None],
                  vbeta=v_beta[:, None], obeta=out_beta[:, None], rmio=rmio)
    if use_vb:
        shared["vbr"] = v_beta[None, :].astype(np.float16)

    prow = np.arange(P, dtype=np.int64)
    in_maps = []
    for r in range(NCORE):
        sl = slice(r * NSH, (r + 1) * NSH)
        slots = idx_new[:, sl]      # [K, NSH] new ids (compacted)
        biasr = bias_pad[:, sl]     # [K, NSH]
        # k-major edge-expanded x for phase A: [128, TO*K*128]
        ja = idx_km[:, sl]          # [K, NSH]
        jlA = ja.reshape(K, TO, P).transpose(1, 0, 2).ravel()  # (t, k, p)
        xeA_r = np.ascontiguousarray(x16g[jlA].T)

        jl_parts = []
        aux_parts = []
        ilo_parts = []
        ihi_parts = []
        code_parts = []
        for t in range(TO):
            KT = kts[t]
            s_tk = slots[:KT, t * P:(t + 1) * P]      # [KT, 128] (k, p)
            b_tk = biasr[:KT, t * P:(t + 1) * P]
            jl_parts.append(s_tk.ravel())             # (k, p) order
            # aux: [128, KT, 5] -> per-partition (k-major) c4 + bias
            a = np.empty((P, KT, 5), np.float32)
            a[:, :, :4] = c4g[s_tk.T]                 # [128, KT, 4]
            a[:, :, 4] = b_tk.T
            aux_parts.append(a.reshape(P, KT * 5))
            # ce lookup tables
            nn = s_tk.T.astype(np.int64)              # [128, KT]
            valid = b_tk.T == 0.0
            fpn = (nn % P) * COLS + nn // P
            slab = fpn // HALFV
            w_in = fpn % HALFV
            ent = w_in // 2 + 1
            m = fpn % 2
            ilo = np.where(slab == 0, ent, 0).astype(np.int16)
            ihi = np.where(slab == 1, ent, 0).astype(np.int16)
            code = np.where(valid, (prow[:, None] % 16) * 2 + m,
                            -1).astype(np.float16)
            ilo_parts.append(np.concatenate([ilo, ihi], axis=1))
            code_parts.append(code)

        jl = np.concatenate(jl_parts)                 # [SUMK*128]
        xeT_r = np.ascontiguousarray(x16g[jl].T)      # [128, SUMK*128]
        aux_r = np.ascontiguousarray(np.concatenate(aux_parts, axis=1))
        pki_r = np.ascontiguousarray(np.concatenate(ilo_parts, axis=1))
        pkc_r = np.ascontiguousarray(np.concatenate(code_parts, axis=1))

        m = dict(shared)
        m["xeA"] = xeA_r
        m["xeT"] = xeT_r
        m["aux"] = aux_r
        m["pki"] = pki_r
        m["pkc"] = pkc_r
        m["xT_own"] = np.ascontiguousarray(xp2[sl].T)
        in_maps.append(m)
    return in_maps, kts, orders, use_bch, use_vb


def prepare(inputs):
    in_maps, kts, orders, use_bch, use_vb = _prep(inputs)
    key = (kts, use_bch, use_vb)
    if _CACHE.get("key") != key:
        _CACHE["nc"] = _build_nc(kts, use_bch, use_vb)
        _CACHE["key"] = key
    return _CACHE["nc"], in_maps, orders


def assemble(results, orders):
    out = np.empty((NCORE * NSH, P), np.float32)
    for r in range(NCORE):
        out[r * NSH + orders[r]] = results[r]["outT"].T
    return np.ascontiguousarray(out[:N])


def kernel(**inputs):
    nc, in_maps, orders = prepare(inputs)
    res = run_bass_kernel_spmd(nc, in_maps, list(range(NCORE)))
    return assemble(res.results, orders)


if __name__ == "__main__":
    rng = np.random.default_rng(0)
    ins = dict(
        x=rng.standard_normal((N, P)).astype(np.float32),
        coords=(rng.random((N, 3)) * 100).astype(np.float32),
        W_q=rng.standard_normal((K, P, VEC)).astype(np.float32) * (P * K) ** -0.5,
        q_gamma=np.ones(VEC, np.float32), q_beta=np.zeros(VEC, np.float32),
        W_v=rng.standard_normal((P, P)).astype(np.float32) * P ** -0.5,
        v_gamma=np.ones(P, np.float32), v_beta=np.zeros(P, np.float32),
        codebook=rng.standard_normal(P).astype(np.float32) * 0.1,
        W_choice=rng.standard_normal((P, P)).astype(np.float32) * P ** -0.5,
        b_choice=np.zeros(P, np.float32),
        W_pos=rng.standard_normal((3, VEC)).astype(np.float32) * 3 ** -0.5,
        b_pos=np.zeros(VEC, np.float32),
        W_out=rng.standard_normal((P, P)).astype(np.float32) * P ** -0.5,
        out_gamma=np.ones(P, np.float32), out_beta=np.zeros(P, np.float32),
        nbr_idx=rng.integers(0, N, (K, N)).astype(np.int32),
        nbr_mask=rng.integers(0, 2, (K, N)).astype(np.int32),
    )
    out = kernel(**ins)
    print("kernel output", out.shape, out.dtype)


# revision 23
# speedup vs baseline: 1.1555x; 1.1555x over previous
"""Trainium2 Bass kernel for nn_DiscreteQKTRBlock (sparse 3x3x3 neighborhood
attention with a discrete codebook).

Strategy (data-parallel over points, 8 cores), v2 "edge-expanded halo":

The discrete-codebook STE path collapses algebraically:
    s[k,i]  = dq[i] . dq[nbr[k,i]] = ||cb||^2 * choice[i] * choice[nbr[k,i]]
so per-offset scores reduce to scalar products of `choice'` = sqrt(cb2)*choice.

Host-side, neighbor indices are fully known, so we pre-expand a "halo" copy of
x per edge slot (xeT, feature-major fp16).  The device then needs NO random
DRAM gathers for x-dependent data:

  A) per consumer tile: q^T = sum_k Wq_k.T @ xe_k  (PSUM accumulation),
     choice' per own point -> strip
  B) AllGather strip (50KB/core); build a per-partition-replicated SBUF table
     of all 100K choice' values (fp16, two 98KB slabs) and resolve per-edge
     neighbor choice via gpsimd ap_gather + diagonal-mask extraction -> ce
  C) per consumer tile: scores = strip*ce + bias, masked softmax; per-slot
     v^T = relu(Wv.T @ xe_k + beta), PE-transpose, weighted DVE accumulation;
     pos is aggregated as sum_k w_k*coords4 and folded through
     (Wpos_exp @ W_out) into the output matmul; relu + residual.

All weight-affine folds are host-side weight-space transforms only.
"""
import sys
sys.path.insert(0, "/opt/trn_rl_repo")
import numpy as np
import ml_dtypes

from concourse import bass, bacc, mybir
import concourse.tile as tile
from concourse.bass_utils import run_bass_kernel_spmd
from concourse.masks import make_identity

F32 = mybir.dt.float32
FP16 = mybir.dt.float16
I16 = mybir.dt.int16
I32 = mybir.dt.int32

N = 100000
P = 128
VEC = 16
K = 27
NEG = -1e9
NCORE = 8
NSH = 12544                 # points per core (98 tiles of 128)
TO = NSH // P               # 98 own tiles
NTOT = NCORE * NSH          # 100352 global (padded) points
Z = N                       # new-id of the guaranteed all-zero pad row
COLS = NCORE * TO           # 784 columns in the wrapped choice layout
HALFV = NTOT // 2           # 50176 choice values per table slab
ENT = HALFV // 2 + 1        # 25089 entries per slab (d=2, incl. zero entry)

_CACHE = {}


def _build_nc(kts, use_bch, use_vb):
    SUMK = sum(kts)
    so = [int(v) for v in np.concatenate([[0], np.cumsum(kts)])]  # slot offsets
    H1 = TO // 2

    nc = bacc.Bacc(num_devices=NCORE, dynamic_dma_scratch_size=16384)

    # ---------------- inputs ----------------
    xeA = nc.declare_dram_parameter("xeA", [P, TO * K * P], FP16, isOutput=False)
    xeT = nc.declare_dram_parameter("xeT", [P, SUMK * P], FP16, isOutput=False)
    aux = nc.declare_dram_parameter("aux", [P, SUMK * 5], F32, isOutput=False)
    pki = nc.declare_dram_parameter("pki", [P, SUMK * 2], I16, isOutput=False)
    pkc = nc.declare_dram_parameter("pkc", [P, SUMK], FP16, isOutput=False)
    xT_own = nc.declare_dram_parameter("xT_own", [P, NSH], F32, isOutput=False)
    w_q = nc.declare_dram_parameter("w_q", [P, K * VEC], FP16, isOutput=False)
    wcc_in = nc.declare_dram_parameter("wcc", [VEC, P], F32, isOutput=False)
    bch_in = nc.declare_dram_parameter("bch", [1, P], F32, isOutput=False)
    wv_in = nc.declare_dram_parameter("wv", [P, P], FP16, isOutput=False)
    wo_in = nc.declare_dram_parameter("wo", [P, P], FP16, isOutput=False)
    wpw_in = nc.declare_dram_parameter("wpw", [4, P], FP16, isOutput=False)
    if use_vb:
        vbr_in = nc.declare_dram_parameter("vbr", [1, P], FP16, isOutput=False)
    qg_in = nc.declare_dram_parameter("qg", [VEC, 1], F32, isOutput=False)
    qb_in = nc.declare_dram_parameter("qb", [VEC, 1], F32, isOutput=False)
    vbeta_in = nc.declare_dram_parameter("vbeta", [P, 1], F32, isOutput=False)
    obeta_in = nc.declare_dram_parameter("obeta", [P, 1], F32, isOutput=False)
    rmio_in = nc.declare_dram_parameter("rmio", [P, 32], FP16, isOutput=False)

    outT = nc.declare_dram_parameter("outT", [P, NSH], F32, isOutput=True)

    AF = mybir.ActivationFunctionType
    ALU = mybir.AluOpType

    with tile.TileContext(nc) as tc:
        with tc.tile_pool(name="persist", bufs=1) as pp, \
             tc.tile_pool(name="dram", bufs=1, space="DRAM") as dpool:
            strip = pp.tile([P, TO], F32)
            qg_sb = pp.tile([VEC, 1], F32)
            nc.sync.dma_start(out=qg_sb[:], in_=qg_in[:, :])
            qb_sb = pp.tile([VEC, 1], F32)
            nc.sync.dma_start(out=qb_sb[:], in_=qb_in[:, :])
            vbeta_sb = pp.tile([P, 1], F32)
            nc.sync.dma_start(out=vbeta_sb[:], in_=vbeta_in[:, :])
            obeta_sb = pp.tile([P, 1], F32)
            nc.sync.dma_start(out=obeta_sb[:], in_=obeta_in[:, :])
            zero_col = pp.tile([P, 1], F32)
            nc.vector.memset(zero_col[:], 0.0)

            c16d = dpool.tile([P, COLS], FP16)
            ced = dpool.tile([P, SUMK], FP16)
            cc_in1 = dpool.tile([P, H1], F32)
            cc_out1 = dpool.tile([NCORE, P, H1], F32, addr_space="Shared")
            cc_in2 = dpool.tile([P, TO - H1], F32)
            cc_out2 = dpool.tile([NCORE, P, TO - H1], F32, addr_space="Shared")

            # ================= scope 1: phase A + allgather =================
            with tc.tile_pool(name="a_const", bufs=1) as acp, \
                 tc.tile_pool(name="a_xe", bufs=2) as axp, \
                 tc.tile_pool(name="a_w", bufs=3) as awp, \
                 tc.tile_pool(name="a_ps", bufs=2, space="PSUM") as apsp, \
                 tc.tile_pool(name="a_ps2", bufs=2, space="PSUM") as apsp2:
                wq_sb = acp.tile([P, K * VEC], FP16)
                nc.sync.dma_start(out=wq_sb[:], in_=w_q[:, :])
                wcc_sb = acp.tile([VEC, P], F32)
                nc.sync.dma_start(out=wcc_sb[:], in_=wcc_in[:, :])
                if use_bch:
                    bch_sb = acp.tile([1, P], F32)
                    nc.sync.dma_start(out=bch_sb[:], in_=bch_in[:, :])
                    ones1 = acp.tile([1, P], F32)
                    nc.vector.memset(ones1[:], 1.0)

                with nc.named_scope("phaseA"):
                    for tg in range(0, TO, 4):
                        nt = min(4, TO - tg)
                        xe4 = axp.tile([P, 4 * K * P], FP16, tag="xe")
                        nc.sync.dma_start(
                            out=xe4[:, 0:nt * K * P],
                            in_=xeA[:, tg * K * P:(tg + nt) * K * P])
                        q4 = apsp.tile([VEC, 4 * P], F32, tag="q",
                                       padded_shape=[P, 4 * P])
                        for k in range(K):
                            rhs = bass.AP(xe4.tensor, xe4[:].offset + k * P,
                                          [xe4[:].ap[0], (K * P, nt), (1, P)])
                            nc.tensor.matmul(
                                out=q4[:, 0:nt * P],
                                lhsT=wq_sb[:, k * VEC:(k + 1) * VEC],
                                rhs=rhs, start=(k == 0), stop=(k == K - 1))
                        qf = awp.tile([VEC, 4 * P], F32, tag="qf")
                        nc.scalar.activation(
                            out=qf[:, 0:nt * P], in_=q4[:, 0:nt * P],
                            func=AF.Relu, bias=qb_sb[:, 0:1],
                            scale=qg_sb[:, 0:1])
                        for j in range(nt):
                            t = tg + j
                            t_ps = apsp2.tile([P, P], F32, tag="t")
                            if use_bch:
                                nc.tensor.matmul(
                                    out=t_ps[:], lhsT=qf[:, j * P:(j + 1) * P],
                                    rhs=wcc_sb[:], start=True, stop=False)
                                nc.tensor.matmul(
                                    out=t_ps[:], lhsT=ones1[:], rhs=bch_sb[:],
                                    start=False, stop=True)
                            else:
                                nc.tensor.matmul(
                                    out=t_ps[:], lhsT=qf[:, j * P:(j + 1) * P],
                                    rhs=wcc_sb[:], start=True, stop=True)
                            scratch = awp.tile([P, P], FP16, tag="scr")
                            nc.scalar.activation(
                                out=scratch[:], in_=t_ps[:], func=AF.Relu,
                                accum_out=strip[:, t:t + 1])

                with nc.named_scope("gather_choice"):
                    nc.sync.dma_start(out=cc_in1[:], in_=strip[:, 0:H1])
                    nc.gpsimd.collective_compute(
                        "AllGather", ALU.bypass,
                        replica_groups=[list(range(NCORE))],
                        ins=[cc_in1.opt()], outs=[cc_out1.opt()])
                    nc.sync.dma_start(out=cc_in2[:], in_=strip[:, H1:TO])
                    nc.gpsimd.collective_compute(
                        "AllGather", ALU.bypass,
                        replica_groups=[list(range(NCORE))],
                        ins=[cc_in2.opt()], outs=[cc_out2.opt()])

            # ================= scope 2a: choice table to DRAM ===============
            with tc.tile_pool(name="b_ch", bufs=1) as bchp:
                with nc.named_scope("chprep"):
                    ch32 = bchp.tile([P, COLS], F32)
                    ca_rt = ch32[:, 0:COLS].rearrange("p (r t) -> p r t", r=NCORE)
                    nc.sync.dma_start(
                        out=ca_rt[:, :, 0:H1],
                        in_=cc_out1[:, :, :].rearrange("r p t -> p r t"))
                    nc.sync.dma_start(
                        out=ca_rt[:, :, H1:TO],
                        in_=cc_out2[:, :, :].rearrange("r p t -> p r t"))
                    ch16 = bchp.tile([P, COLS], FP16)
                    nc.vector.tensor_copy(out=ch16[:], in_=ch32[:])
                    nc.sync.dma_start(out=c16d[:, :], in_=ch16[:])

            # ================= scope 2b: per-edge choice (ce) ===============
            with tc.tile_pool(name="c_fix", bufs=1) as cfp, \
                 tc.tile_pool(name="c_tab", bufs=1) as ctp, \
                 tc.tile_pool(name="c_pk", bufs=2) as cpkp, \
                 tc.tile_pool(name="c_raw", bufs=2) as crawp, \
                 tc.tile_pool(name="c_w", bufs=2) as cwp:
                rm_sb = cfp.tile([P, 32], FP16)
                nc.sync.dma_start(out=rm_sb[:], in_=rmio_in[:, :])
                celo = cfp.tile([P, SUMK], F32)


                with nc.named_scope("cepass"):
                    for s in range(2):
                        tab = ctp.tile([P, 2 * ENT], FP16, tag="tab")
                        nc.vector.memset(tab[:, 0:2], 0.0)
                        src = bass.AP(c16d.tensor, s * HALFV,
                                      [(0, P), (1, HALFV)])
                        nc.sync.dma_start(out=tab[:, 2:2 + HALFV], in_=src)
                        for t in range(TO):
                            KT = kts[t]
                            pki_t = cpkp.tile([P, KT], I16, tag="pki")
                            nc.sync.dma_start(
                                out=pki_t[:],
                                in_=pki[:, so[t] * 2 + s * KT:
                                        so[t] * 2 + (s + 1) * KT])
                            code_t = cpkp.tile([P, KT], FP16, tag="pkc")
                            nc.scalar.dma_start(
                                out=code_t[:], in_=pkc[:, so[t]:so[t] + KT])
                            raw = crawp.tile([P, 16 * KT * 2], FP16, tag="raw")
                            nc.gpsimd.ap_gather(
                                out_ap=raw[:].rearrange("p (n d) -> p n d", d=2),
                                in_ap=tab[:].rearrange("p (n d) -> p n d", d=2),
                                idxs_ap=pki_t[:, 0:KT],
                                channels=P, num_elems=ENT, d=2,
                                num_idxs=16 * KT)
                            mask = cwp.tile([P, KT * 32], FP16, tag="mk")
                            code_bc = bass.AP(code_t.tensor, code_t[:].offset,
                                              [code_t[:].ap[0], (1, KT),
                                               (0, 32)])
                            rm_bc = bass.AP(rm_sb.tensor, rm_sb[:].offset,
                                            [rm_sb[:].ap[0], (0, KT), (1, 32)])
                            nc.vector.tensor_tensor(
                                out=mask[:].rearrange("p (a b) -> p a b", b=32),
                                in0=code_bc, in1=rm_bc, op=ALU.is_equal)
                            prod = cwp.tile([P, KT * 32], FP16, tag="pr")
                            nc.vector.tensor_tensor(
                                out=prod[:], in0=raw[:], in1=mask[:],
                                op=ALU.mult)
                            if s == 0:
                                nc.vector.tensor_reduce(
                                    out=celo[:, so[t]:so[t] + KT],
                                    in_=prod[:].rearrange(
                                        "p (a b) -> p a b", b=32),
                                    axis=mybir.AxisListType.X, op=ALU.add)
                            else:
                                cet = cwp.tile([P, KT], F32, tag="cet")
                                nc.vector.tensor_reduce(
                                    out=cet[:],
                                    in_=prod[:].rearrange(
                                        "p (a b) -> p a b", b=32),
                                    axis=mybir.AxisListType.X, op=ALU.add)
                                ce16 = cwp.tile([P, KT], FP16, tag="ce16")
                                nc.vector.tensor_tensor(
                                    out=ce16[:], in0=cet[:],
                                    in1=celo[:, so[t]:so[t] + KT], op=ALU.add)
                                nc.scalar.dma_start(
                                    out=ced[:, so[t]:so[t] + KT], in_=ce16[:])

            # ================= scope 3: phase C =============================
            with tc.tile_pool(name="d_const", bufs=1) as dcp, \
                 tc.tile_pool(name="d_xe", bufs=3) as dxp, \
                 tc.tile_pool(name="d_aux", bufs=2) as dauxp, \
                 tc.tile_pool(name="d_w", bufs=3) as dwp, \
                 tc.tile_pool(name="d_vps", bufs=3, space="PSUM") as dvps, \
                 tc.tile_pool(name="d_tps", bufs=2, space="PSUM") as dtps, \
                 tc.tile_pool(name="d_t1ps", bufs=1, space="PSUM") as dt1ps, \
                 tc.tile_pool(name="d_ops", bufs=1, space="PSUM") as dops:
                wv_sb = dcp.tile([P, P], FP16)
                nc.sync.dma_start(out=wv_sb[:], in_=wv_in[:, :])
                wo_sb = dcp.tile([P, P], FP16)
                nc.sync.dma_start(out=wo_sb[:], in_=wo_in[:, :])
                wpw_sb = dcp.tile([4, P], FP16)
                nc.sync.dma_start(out=wpw_sb[:], in_=wpw_in[:, :])
                ident16 = dcp.tile([P, P], FP16)
                make_identity(nc, ident16[:])
                aux_sb = dcp.tile([P, SUMK * 5], F32)
                nc.sync.dma_start(out=aux_sb[:], in_=aux[:, :])
                if use_vb:
                    vbr_sb = dcp.tile([1, P], FP16)
                    nc.sync.dma_start(out=vbr_sb[:], in_=vbr_in[:, :])
                    ones1f = dcp.tile([1, P], FP16)
                    nc.vector.memset(ones1f[:], 1.0)

                with nc.named_scope("phaseC"):
                    for t in range(TO):
                        KT = kts[t]
                        xe_t = dxp.tile([P, KT * P], FP16, tag="xe")
                        nc.sync.dma_start(
                            out=xe_t[:], in_=xeT[:, so[t] * P:(so[t] + KT) * P])
                        ce_t = dauxp.tile([P, KT], FP16, tag="ce")
                        nc.scalar.dma_start(
                            out=ce_t[:], in_=ced[:, so[t]:so[t] + KT])
                        xo_t = dauxp.tile([P, P], F32, tag="xo")
                        nc.sync.dma_start(
                            out=xo_t[:], in_=xT_own[:, t * P:(t + 1) * P])

                        # scores + masked softmax
                        s_t = dwp.tile([P, KT], F32, tag="s")
                        bias_view = bass.AP(aux_sb.tensor,
                                            aux_sb[:].offset + so[t] * 5 + 4,
                                            [aux_sb[:].ap[0], (5, KT)])
                        nc.vector.scalar_tensor_tensor(
                            out=s_t[:], in0=ce_t[:], scalar=strip[:, t:t + 1],
                            in1=bias_view, op0=ALU.mult, op1=ALU.add)
                        negmax = dwp.tile([P, 1], F32, tag="nm")
                        nc.vector.tensor_reduce(
                            out=negmax[:], in_=s_t[:], axis=mybir.AxisListType.X,
                            op=ALU.max, negate=True)
                        e_t = dwp.tile([P, KT], F32, tag="e")
                        esum = dwp.tile([P, 1], F32, tag="es")
                        nc.scalar.activation(
                            out=e_t[:], in_=s_t[:], func=AF.Exp,
                            bias=negmax[:, 0:1], scale=1.0,
                            accum_out=esum[:, 0:1])
                        rs = dwp.tile([P, 1], F32, tag="rsx")
                        nc.vector.reciprocal(out=rs[:], in_=esum[:])
                        w_t = dwp.tile([P, KT], F32, tag="w")
                        nc.vector.tensor_scalar_mul(out=w_t[:], in0=e_t[:],
                                                    scalar1=rs[:, 0:1])

                        # pos: aggregate coords4 with attn weights
                        c4_view = bass.AP(aux_sb.tensor,
                                          aux_sb[:].offset + so[t] * 5,
                                          [aux_sb[:].ap[0], (5, KT), (1, 4)])
                        w_bc = bass.AP(w_t.tensor, w_t[:].offset,
                                       [w_t[:].ap[0], (1, KT), (0, 4)])
                        tmp4 = dwp.tile([P, KT * 4], F32, tag="t4")
                        nc.vector.tensor_tensor(
                            out=tmp4[:].rearrange("p (a b) -> p a b", b=4),
                            in0=c4_view, in1=w_bc, op=ALU.mult)
                        ag4 = dwp.tile([P, 4], F32, tag="a4")
                        ag4_in = bass.AP(tmp4.tensor, tmp4[:].offset,
                                         [tmp4[:].ap[0], (1, 4), (4, KT)])
                        nc.vector.tensor_reduce(
                            out=ag4[:], in_=ag4_in, axis=mybir.AxisListType.X,
                            op=ALU.add)
                        ag416 = dwp.tile([P, 4], FP16, tag="a416")
                        nc.scalar.copy(out=ag416[:], in_=ag4[:])
                        a4T_ps = dt1ps.tile([4, P], FP16, tag="a4T",
                                            padded_shape=[P, P])
                        nc.tensor.transpose(out=a4T_ps[:], in_=ag416[:],
                                            identity=ident16[:])
                        a4T = dwp.tile([4, P], FP16, tag="a4Ts")
                        nc.scalar.copy(out=a4T[:], in_=a4T_ps[:])

                        # weighted aggregation of v (points on out partitions)
                        accA = dwp.tile([P, P], FP16, tag="accA")
                        accB = dwp.tile([P, P], FP16, tag="accB")
                        for k0 in range(0, KT, 4):
                            nk = min(4, KT - k0)
                            v4 = dvps.tile([P, 4 * P], F32, tag="v")
                            for j in range(nk):
                                if use_vb:
                                    nc.tensor.matmul(
                                        out=v4[:, j * P:(j + 1) * P],
                                        lhsT=xe_t[:, (k0 + j) * P:
                                                  (k0 + j + 1) * P],
                                        rhs=wv_sb[:], start=True, stop=False)
                                    nc.tensor.matmul(
                                        out=v4[:, j * P:(j + 1) * P],
                                        lhsT=ones1f[:], rhs=vbr_sb[:],
                                        start=False, stop=True)
                                else:
                                    nc.tensor.matmul(
                                        out=v4[:, j * P:(j + 1) * P],
                                        lhsT=xe_t[:, (k0 + j) * P:
                                                  (k0 + j + 1) * P],
                                        rhs=wv_sb[:], start=True, stop=True)
                            vT4 = dwp.tile([P, 4 * P], FP16, tag="vT")
                            if (k0 // 4) % 2 == 0:
                                nc.scalar.activation(
                                    out=vT4[:, 0:nk * P], in_=v4[:, 0:nk * P],
                                    func=AF.Relu)
                            else:
                                nc.vector.tensor_scalar_max(
                                    out=vT4[:, 0:nk * P], in0=v4[:, 0:nk * P],
                                    scalar1=0.0)
                            for j in range(nk):
                                k = k0 + j
                                sl = vT4[:, j * P:(j + 1) * P]
                                wk = w_t[:, k:k + 1]
                                if k == 0:
                                    nc.vector.tensor_scalar_mul(
                                        out=accA[:], in0=sl, scalar1=wk)
                                elif k == 1:
                                    nc.vector.tensor_scalar_mul(
                                        out=accB[:], in0=sl, scalar1=wk)
                                elif k % 2 == 0:
                                    nc.vector.scalar_tensor_tensor(
                                        out=accA[:], in0=sl, scalar=wk,
                                        op0=ALU.mult, in1=accA[:], op1=ALU.add)
                                else:
                                    nc.vector.scalar_tensor_tensor(
                                        out=accB[:], in0=sl, scalar=wk,
                                        op0=ALU.mult, in1=accB[:], op1=ALU.add)
                        acc = dwp.tile([P, P], FP16, tag="acc")
                        if KT == 1:
                            nc.vector.tensor_copy(out=acc[:], in_=accA[:])
                        else:
                            nc.vector.tensor_tensor(
                                out=acc[:], in0=accA[:], in1=accB[:],
                                op=ALU.add)

                        accT_ps = dt1ps.tile([P, P], FP16, tag="accT")
                        nc.tensor.transpose(out=accT_ps[:], in_=acc[:],
                                            identity=ident16[:])
                        accT = dwp.tile([P, P], FP16, tag="accTs")
                        nc.scalar.copy(out=accT[:], in_=accT_ps[:])
                        o_ps = dops.tile([P, P], F32, tag="o")
                        nc.tensor.matmul(out=o_ps[:], lhsT=wo_sb[:], rhs=accT[:],
                                         start=True, stop=False)
                        nc.tensor.matmul(out=o_ps[:], lhsT=wpw_sb[:], rhs=a4T[:],
                                         start=False, stop=True)
                        oT = dwp.tile([P, P], F32, tag="oT")
                        nc.scalar.activation(
                            out=oT[:], in_=o_ps[:], func=AF.Relu,
                            bias=obeta_sb[:, 0:1])
                        res = dwp.tile([P, P], F32, tag="res")
                        nc.vector.tensor_tensor(out=res[:], in0=oT[:],
                                                in1=xo_t[:], op=ALU.add)
                        nc.sync.dma_start(out=outT[:, t * P:(t + 1) * P],
                                          in_=res[:])

    nc.finalize()
    return nc


def _prep(inputs):
    x = np.asarray(inputs["x"], np.float32)
    coords = np.asarray(inputs["coords"], np.float32)
    W_q = np.asarray(inputs["W_q"], np.float32)
    q_gamma = np.asarray(inputs["q_gamma"], np.float32)
    q_beta = np.asarray(inputs["q_beta"], np.float32)
    W_v = np.asarray(inputs["W_v"], np.float32)
    v_gamma = np.asarray(inputs["v_gamma"], np.float32)
    v_beta = np.asarray(inputs["v_beta"], np.float32)
    codebook = np.asarray(inputs["codebook"], np.float32)
    W_choice = np.asarray(inputs["W_choice"], np.float32)
    b_choice = np.asarray(inputs["b_choice"], np.float32)
    W_pos = np.asarray(inputs["W_pos"], np.float32)
    b_pos = np.asarray(inputs["b_pos"], np.float32)
    W_out = np.asarray(inputs["W_out"], np.float32)
    out_gamma = np.asarray(inputs["out_gamma"], np.float32)
    out_beta = np.asarray(inputs["out_beta"], np.float32)
    nbr_idx = np.asarray(inputs["nbr_idx"], np.int32)
    nbr_mask = np.asarray(inputs["nbr_mask"], np.int32)

    n = x.shape[0]
    assert n == N

    # ---- valid-degree sort (per core shard) -> global relabeling ----
    mask_pad = np.zeros((K, NTOT), bool)
    mask_pad[:, :n] = nbr_mask > 0
    deg = mask_pad.sum(0)
    orders = []
    degs_sorted = np.empty((NCORE, NSH), np.int64)
    for r in range(NCORE):
        sl = slice(r * NSH, (r + 1) * NSH)
        o = np.argsort(-deg[sl], kind="stable")
        orders.append(o)
        degs_sorted[r] = deg[sl][o]
    kts = tuple(int(max(1, degs_sorted[:, t * P:(t + 1) * P].max()))
                for t in range(TO))
    SUMK = sum(kts)
    perm_full = np.concatenate([r * NSH + orders[r] for r in range(NCORE)])
    inv = np.empty(NTOT, np.int64)
    inv[perm_full] = np.arange(NTOT)

    # ---- permuted global tables (new-id order) ----
    xp = np.zeros((NTOT, P), np.float32)
    xp[:n] = x
    xp2 = xp[perm_full]
    x16g = xp2.astype(np.float16)
    cp = np.zeros((NTOT, 3), np.float32)
    cp[:n] = coords
    c4g = np.ones((NTOT, 4), np.float32)
    c4g[:, :3] = cp[perm_full]

    # ---- weight folds ----
    cb2 = float(np.dot(codebook, codebook))
    scb = np.sqrt(cb2).astype(np.float32)
    wcp = codebook[:, None] * W_choice
    wcc = scb * wcp.reshape(VEC, P // VEC, P).sum(1)
    bch = (scb * b_choice)[None, :]
    use_bch = bool(np.any(b_choice != 0))
    wq_flat = np.ascontiguousarray(
        W_q.transpose(1, 0, 2).reshape(P, K * VEC)).astype(np.float16)
    wv16 = (W_v * v_gamma[None, :]).astype(np.float16)
    use_vb = bool(np.any(v_beta != 0))
    wo = W_out * out_gamma[None, :]
    wo16 = wo.astype(np.float16)
    woB = wo.reshape(VEC, P // VEC, P).sum(1)          # [16, 128]
    wpos4 = np.concatenate([W_pos, b_pos[None, :]], axis=0)  # [4, 16]
    wpw16 = (wpos4 @ woB).astype(np.float16)           # [4, 128]
    rmio = np.tile(np.arange(32, dtype=np.float16)[None, :], (P, 1))

    # ---- per-slot neighbor ids (new ids, valid-first compaction) ----
    idx_new = np.full((K, NTOT), Z, np.int32)
    idx_new[:, :n] = np.where(nbr_mask > 0, inv[nbr_idx], Z).astype(np.int32)
    bias_pad = np.full((K, NTOT), np.float32(NEG), np.float32)
    bias_pad[:, :n] = np.where(nbr_mask > 0, 0.0, NEG).astype(np.float32)
    idx_km = idx_new[:, perm_full]          # k-major (original offsets)
    korder = np.argsort(~mask_pad, axis=0, kind="stable")   # valid ks first
    idx_new = np.take_along_axis(idx_new, korder, axis=0)
    bias_pad = np.take_along_axis(bias_pad, korder, axis=0)
    # permute slot-grid columns to sorted point order
    idx_new = idx_new[:, perm_full]
    bias_pad = bias_pad[:, perm_full]

    shared = dict(w_q=wq_flat, wcc=wcc, bch=bch, wv=wv16, wo=wo16,
                  wpw=wpw16, qg=q_gamma[:, None], qb=q_beta[:, None],
                  vbeta=v_beta[:, None], obeta=out_beta[:, None], rmio=rmio)
    if use_vb:
        shared["vbr"] = v_beta[None, :].astype(np.float16)

    prow = np.arange(P, dtype=np.int64)
    in_maps = []
    for r in range(NCORE):
        sl = slice(r * NSH, (r + 1) * NSH)
        slots = idx_new[:, sl]      # [K, NSH] new ids (compacted)
        biasr = bias_pad[:, sl]     # [K, NSH]
        # k-major edge-expanded x for phase A: [128, TO*K*128]
        ja = idx_km[:, sl]          # [K, NSH]
        jlA = ja.reshape(K, TO, P).transpose(1, 0, 2).ravel()  # (t, k, p)
        xeA_r = np.ascontiguousarray(x16g[jlA].T)

        jl_parts = []
        aux_parts = []
        ilo_parts = []
        ihi_parts = []
        code_parts = []
        for t in range(TO):
            KT = kts[t]
            s_tk = slots[:KT, t * P:(t + 1) * P]      # [KT, 128] (k, p)
            b_tk = biasr[:KT, t * P:(t + 1) * P]
            jl_parts.append(s_tk.ravel())             # (k, p) order
            # aux: [128, KT, 5] -> per-partition (k-major) c4 + bias
            a = np.empty((P, KT, 5), np.float32)
            a[:, :, :4] = c4g[s_tk.T]                 # [128, KT, 4]
            a[:, :, 4] = b_tk.T
            aux_parts.append(a.reshape(P, KT * 5))
            # ce lookup tables
            nn = s_tk.T.astype(np.int64)              # [128, KT]
            valid = b_tk.T == 0.0
            fpn = (nn % P) * COLS + nn // P
            slab = fpn // HALFV
            w_in = fpn % HALFV
            ent = w_in // 2 + 1
            m = fpn % 2
            ilo = np.where(slab == 0, ent, 0).astype(np.int16)
            ihi = np.where(slab == 1, ent, 0).astype(np.int16)
            code = np.where(valid, (prow[:, None] % 16) * 2 + m,
                            -1).astype(np.float16)
            ilo_parts.append(np.concatenate([ilo, ihi], axis=1))
            code_parts.append(code)

        jl = np.concatenate(jl_parts)                 # [SUMK*128]
        xeT_r = np.ascontiguousarray(x16g[jl].T)      # [128, SUMK*128]
        aux_r = np.ascontiguousarray(np.concatenate(aux_parts, axis=1))
        pki_r = np.ascontiguousarray(np.concatenate(ilo_parts, axis=1))
        pkc_r = np.ascontiguousarray(np.concatenate(code_parts, axis=1))

        m = dict(shared)
        m["xeA"] = xeA_r
        m["xeT"] = xeT_r
        m["aux"] = aux_r
        m["pki"] = pki_r
        m["pkc"] = pkc_r
        m["xT_own"] = np.ascontiguousarray(xp2[sl].T)
        in_maps.append(m)
    return in_maps, kts, orders, use_bch, use_vb


def prepare(inputs):
    in_maps, kts, orders, use_bch, use_vb = _prep(inputs)
    key = (kts, use_bch, use_vb)
    if _CACHE.get("key") != key:
        _CACHE["nc"] = _build_nc(kts, use_bch, use_vb)
        _CACHE["key"] = key
    return _CACHE["nc"], in_maps, orders


def assemble(results, orders):
    out = np.empty((NCORE * NSH, P), np.float32)
    for r in range(NCORE):
        out[r * NSH + orders[r]] = results[r]["outT"].T
    return np.ascontiguousarray(out[:N])


def kernel(**inputs):
    nc, in_maps, orders = prepare(inputs)
    res = run_bass_kernel_spmd(nc, in_maps, list(range(NCORE)))
    return assemble(res.results, orders)


if __name__ == "__main__":
    rng = np.random.default_rng(0)
    ins = dict(
        x=rng.standard_normal((N, P)).astype(np.float32),
        coords=(rng.random((N, 3)) * 100).astype(np.float32),
        W_q=rng.standard_normal((K, P, VEC)).astype(np.float32) * (P * K) ** -0.5,
        q_gamma=np.ones(VEC, np.float32), q_beta=np.zeros(VEC, np.float32),
        W_v=rng.standard_normal((P, P)).astype(np.float32) * P ** -0.5,
        v_gamma=np.ones(P, np.float32), v_beta=np.zeros(P, np.float32),
        codebook=rng.standard_normal(P).astype(np.float32) * 0.1,
        W_choice=rng.standard_normal((P, P)).astype(np.float32) * P ** -0.5,
        b_choice=np.zeros(P, np.float32),
        W_pos=rng.standard_normal((3, VEC)).astype(np.float32) * 3 ** -0.5,
        b_pos=np.zeros(VEC, np.float32),
        W_out=rng.standard_normal((P, P)).astype(np.float32) * P ** -0.5,
        out_gamma=np.ones(P, np.float32), out_beta=np.zeros(P, np.float32),
        nbr_idx=rng.integers(0, N, (K, N)).astype(np.int32),
        nbr_mask=rng.integers(0, 2, (K, N)).astype(np.int32),
    )
    out = kernel(**ins)
    print("kernel output", out.shape, out.dtype)


# revision 26
# speedup vs baseline: 1.1821x; 1.0230x over previous
"""Trainium2 Bass kernel for nn_DiscreteQKTRBlock (sparse 3x3x3 neighborhood
attention with a discrete codebook).

Strategy (data-parallel over points, 8 cores), v2 "edge-expanded halo":

The discrete-codebook STE path collapses algebraically:
    s[k,i]  = dq[i] . dq[nbr[k,i]] = ||cb||^2 * choice[i] * choice[nbr[k,i]]
so per-offset scores reduce to scalar products of `choice'` = sqrt(cb2)*choice.

Host-side, neighbor indices are fully known, so we pre-expand a "halo" copy of
x per edge slot (xeT, feature-major fp16).  The device then needs NO random
DRAM gathers for x-dependent data:

  A) per consumer tile: q^T = sum_k Wq_k.T @ xe_k  (PSUM accumulation),
     choice' per own point -> strip
  B) AllGather strip (50KB/core); build a per-partition-replicated SBUF table
     of all 100K choice' values (fp16, two 98KB slabs) and resolve per-edge
     neighbor choice via gpsimd ap_gather + diagonal-mask extraction -> ce
  C) per consumer tile: scores = strip*ce + bias, masked softmax; per-slot
     v^T = relu(Wv.T @ xe_k + beta), PE-transpose, weighted DVE accumulation;
     pos is aggregated as sum_k w_k*coords4 and folded through
     (Wpos_exp @ W_out) into the output matmul; relu + residual.

All weight-affine folds are host-side weight-space transforms only.
"""
import sys
sys.path.insert(0, "/opt/trn_rl_repo")
import numpy as np
import ml_dtypes

from concourse import bass, bacc, mybir
import concourse.tile as tile
from concourse.bass_utils import run_bass_kernel_spmd
from concourse.masks import make_identity

F32 = mybir.dt.float32
FP16 = mybir.dt.float16
I16 = mybir.dt.int16
I32 = mybir.dt.int32

N = 100000
P = 128
VEC = 16
K = 27
NEG = -1e9
NCORE = 8
NSH = 12544                 # points per core (98 tiles of 128)
TO = NSH // P               # 98 own tiles
NTOT = NCORE * NSH          # 100352 global (padded) points
Z = N                       # new-id of the guaranteed all-zero pad row
COLS = NCORE * TO           # 784 columns in the wrapped choice layout
HALFV = NTOT // 2           # 50176 choice values per table slab
ENT = HALFV // 2 + 1        # 25089 entries per slab (d=2, incl. zero entry)

_CACHE = {}


def _build_nc(kts, use_bch, use_vb):
    SUMK = sum(kts)
    so = [int(v) for v in np.concatenate([[0], np.cumsum(kts)])]  # slot offsets
    H1 = TO // 2

    nc = bacc.Bacc(num_devices=NCORE, dynamic_dma_scratch_size=16384)

    # ---------------- inputs ----------------
    xeA = nc.declare_dram_parameter("xeA", [P, TO * K * P], FP16, isOutput=False)
    xeT = nc.declare_dram_parameter("xeT", [P, SUMK * P], FP16, isOutput=False)
    aux = nc.declare_dram_parameter("aux", [P, SUMK * 5], F32, isOutput=False)
    pki = nc.declare_dram_parameter("pki", [P, SUMK * 2], I16, isOutput=False)
    pkc = nc.declare_dram_parameter("pkc", [P, SUMK], FP16, isOutput=False)
    xT_own = nc.declare_dram_parameter("xT_own", [P, NSH], F32, isOutput=False)
    w_q = nc.declare_dram_parameter("w_q", [P, K * VEC], FP16, isOutput=False)
    wcc_in = nc.declare_dram_parameter("wcc", [VEC, P], F32, isOutput=False)
    bch_in = nc.declare_dram_parameter("bch", [1, P], F32, isOutput=False)
    wv_in = nc.declare_dram_parameter("wv", [P, P], FP16, isOutput=False)
    wo_in = nc.declare_dram_parameter("wo", [P, P], FP16, isOutput=False)
    wpw_in = nc.declare_dram_parameter("wpw", [4, P], FP16, isOutput=False)
    if use_vb:
        vbr_in = nc.declare_dram_parameter("vbr", [1, P], FP16, isOutput=False)
    qg_in = nc.declare_dram_parameter("qg", [VEC, 1], F32, isOutput=False)
    qb_in = nc.declare_dram_parameter("qb", [VEC, 1], F32, isOutput=False)
    vbeta_in = nc.declare_dram_parameter("vbeta", [P, 1], F32, isOutput=False)
    obeta_in = nc.declare_dram_parameter("obeta", [P, 1], F32, isOutput=False)
    rmio_in = nc.declare_dram_parameter("rmio", [P, 32], FP16, isOutput=False)

    outT = nc.declare_dram_parameter("outT", [P, NSH], F32, isOutput=True)

    AF = mybir.ActivationFunctionType
    ALU = mybir.AluOpType

    with tile.TileContext(nc) as tc:
        with tc.tile_pool(name="persist", bufs=1) as pp, \
             tc.tile_pool(name="dram", bufs=1, space="DRAM") as dpool:
            strip = pp.tile([P, TO], F32)
            qg_sb = pp.tile([VEC, 1], F32)
            nc.sync.dma_start(out=qg_sb[:], in_=qg_in[:, :])
            qb_sb = pp.tile([VEC, 1], F32)
            nc.sync.dma_start(out=qb_sb[:], in_=qb_in[:, :])
            vbeta_sb = pp.tile([P, 1], F32)
            nc.sync.dma_start(out=vbeta_sb[:], in_=vbeta_in[:, :])
            obeta_sb = pp.tile([P, 1], F32)
            nc.sync.dma_start(out=obeta_sb[:], in_=obeta_in[:, :])
            zero_col = pp.tile([P, 1], F32)
            nc.vector.memset(zero_col[:], 0.0)
            ce_all = pp.tile([P, SUMK], FP16)

            c16d = dpool.tile([P, COLS], FP16)
            ced = dpool.tile([P, SUMK], FP16)
            cc_in1 = dpool.tile([P, H1], F32)
            cc_out1 = dpool.tile([NCORE, P, H1], F32, addr_space="Shared")
            cc_in2 = dpool.tile([P, TO - H1], F32)
            cc_out2 = dpool.tile([NCORE, P, TO - H1], F32, addr_space="Shared")

            # ================= scope 1: phase A + allgather =================
            with tc.tile_pool(name="a_const", bufs=1) as acp, \
                 tc.tile_pool(name="a_xe", bufs=2) as axp, \
                 tc.tile_pool(name="a_w", bufs=3) as awp, \
                 tc.tile_pool(name="a_ps", bufs=2, space="PSUM") as apsp, \
                 tc.tile_pool(name="a_ps2", bufs=2, space="PSUM") as apsp2:
                wq_sb = acp.tile([P, K * VEC], FP16)
                nc.sync.dma_start(out=wq_sb[:], in_=w_q[:, :])
                wcc_sb = acp.tile([VEC, P], F32)
                nc.sync.dma_start(out=wcc_sb[:], in_=wcc_in[:, :])
                if use_bch:
                    bch_sb = acp.tile([1, P], F32)
                    nc.sync.dma_start(out=bch_sb[:], in_=bch_in[:, :])
                    ones1 = acp.tile([1, P], F32)
                    nc.vector.memset(ones1[:], 1.0)

                with nc.named_scope("phaseA"):
                    for tg in range(0, TO, 4):
                        nt = min(4, TO - tg)
                        xe4 = axp.tile([P, 4 * K * P], FP16, tag="xe")
                        nc.sync.dma_start(
                            out=xe4[:, 0:nt * K * P],
                            in_=xeA[:, tg * K * P:(tg + nt) * K * P])
                        q4 = apsp.tile([VEC, 4 * P], F32, tag="q",
                                       padded_shape=[P, 4 * P])
                        for k in range(K):
                            rhs = bass.AP(xe4.tensor, xe4[:].offset + k * P,
                                          [xe4[:].ap[0], (K * P, nt), (1, P)])
                            nc.tensor.matmul(
                                out=q4[:, 0:nt * P],
                                lhsT=wq_sb[:, k * VEC:(k + 1) * VEC],
                                rhs=rhs, start=(k == 0), stop=(k == K - 1))
                        qf = awp.tile([VEC, 4 * P], F32, tag="qf")
                        nc.scalar.activation(
                            out=qf[:, 0:nt * P], in_=q4[:, 0:nt * P],
                            func=AF.Relu, bias=qb_sb[:, 0:1],
                            scale=qg_sb[:, 0:1])
                        for j in range(nt):
                            t = tg + j
                            t_ps = apsp2.tile([P, P], F32, tag="t")
                            if use_bch:
                                nc.tensor.matmul(
                                    out=t_ps[:], lhsT=qf[:, j * P:(j + 1) * P],
                                    rhs=wcc_sb[:], start=True, stop=False)
                                nc.tensor.matmul(
                                    out=t_ps[:], lhsT=ones1[:], rhs=bch_sb[:],
                                    start=False, stop=True)
                            else:
                                nc.tensor.matmul(
                                    out=t_ps[:], lhsT=qf[:, j * P:(j + 1) * P],
                                    rhs=wcc_sb[:], start=True, stop=True)
                            scratch = awp.tile([P, P], FP16, tag="scr")
                            nc.scalar.activation(
                                out=scratch[:], in_=t_ps[:], func=AF.Relu,
                                accum_out=strip[:, t:t + 1])

                with nc.named_scope("gather_choice"):
                    nc.sync.dma_start(out=cc_in1[:], in_=strip[:, 0:H1])
                    nc.gpsimd.collective_compute(
                        "AllGather", ALU.bypass,
                        replica_groups=[list(range(NCORE))],
                        ins=[cc_in1.opt()], outs=[cc_out1.opt()])
                    nc.sync.dma_start(out=cc_in2[:], in_=strip[:, H1:TO])
                    nc.gpsimd.collective_compute(
                        "AllGather", ALU.bypass,
                        replica_groups=[list(range(NCORE))],
                        ins=[cc_in2.opt()], outs=[cc_out2.opt()])

            # ================= scope 2a: choice table to DRAM ===============
            with tc.tile_pool(name="b_ch", bufs=1) as bchp:
                with nc.named_scope("chprep"):
                    ch32 = bchp.tile([P, COLS], F32)
                    ca_rt = ch32[:, 0:COLS].rearrange("p (r t) -> p r t", r=NCORE)
                    nc.sync.dma_start(
                        out=ca_rt[:, :, 0:H1],
                        in_=cc_out1[:, :, :].rearrange("r p t -> p r t"))
                    nc.sync.dma_start(
                        out=ca_rt[:, :, H1:TO],
                        in_=cc_out2[:, :, :].rearrange("r p t -> p r t"))
                    ch16 = bchp.tile([P, COLS], FP16)
                    nc.vector.tensor_copy(out=ch16[:], in_=ch32[:])
                    nc.sync.dma_start(out=c16d[:, :], in_=ch16[:])

            # ================= scope 2b: per-edge choice (ce) ===============
            with tc.tile_pool(name="c_fix", bufs=1) as cfp, \
                 tc.tile_pool(name="c_tab", bufs=1) as ctp, \
                 tc.tile_pool(name="c_pk", bufs=2) as cpkp, \
                 tc.tile_pool(name="c_raw", bufs=2) as crawp, \
                 tc.tile_pool(name="c_w", bufs=2) as cwp:
                rm_sb = cfp.tile([P, 32], FP16)
                nc.sync.dma_start(out=rm_sb[:], in_=rmio_in[:, :])
                celo = cfp.tile([P, SUMK], F32)
                pki_sb = cfp.tile([P, SUMK * 2], I16)
                nc.sync.dma_start(out=pki_sb[:], in_=pki[:, :])
                pkc_sb = cfp.tile([P, SUMK], FP16)
                nc.scalar.dma_start(out=pkc_sb[:], in_=pkc[:, :])


                with nc.named_scope("cepass"):
                    for s in range(2):
                        tab = ctp.tile([P, 2 * ENT], FP16, tag="tab")
                        nc.vector.memset(tab[:, 0:2], 0.0)
                        src = bass.AP(c16d.tensor, s * HALFV,
                                      [(0, P), (1, HALFV)])
                        nc.sync.dma_start(out=tab[:, 2:2 + HALFV], in_=src)
                        for t in range(TO):
                            KT = kts[t]
                            pki_t = cpkp.tile([P, KT], I16, tag="pki")
                            nc.vector.tensor_copy(
                                out=pki_t[:].bitcast(FP16),
                                in_=pki_sb[:, so[t] * 2 + s * KT:
                                           so[t] * 2 + (s + 1) * KT
                                           ].bitcast(FP16))
                            code_t = cpkp.tile([P, KT], FP16, tag="pkc")
                            nc.scalar.copy(
                                out=code_t[:],
                                in_=pkc_sb[:, so[t]:so[t] + KT])
                            raw = crawp.tile([P, 16 * KT * 2], FP16, tag="raw")
                            nc.gpsimd.ap_gather(
                                out_ap=raw[:].rearrange("p (n d) -> p n d", d=2),
                                in_ap=tab[:].rearrange("p (n d) -> p n d", d=2),
                                idxs_ap=pki_t[:, 0:KT],
                                channels=P, num_elems=ENT, d=2,
                                num_idxs=16 * KT)
                            mask = cwp.tile([P, KT * 32], FP16, tag="mk")
                            code_bc = bass.AP(code_t.tensor, code_t[:].offset,
                                              [code_t[:].ap[0], (1, KT),
                                               (0, 32)])
                            rm_bc = bass.AP(rm_sb.tensor, rm_sb[:].offset,
                                            [rm_sb[:].ap[0], (0, KT), (1, 32)])
                            nc.vector.tensor_tensor(
                                out=mask[:].rearrange("p (a b) -> p a b", b=32),
                                in0=code_bc, in1=rm_bc, op=ALU.is_equal)
                            prod = cwp.tile([P, KT * 32], FP16, tag="pr")
                            nc.vector.tensor_tensor(
                                out=prod[:], in0=raw[:], in1=mask[:],
                                op=ALU.mult)
                            if s == 0:
                                nc.vector.tensor_reduce(
                                    out=celo[:, so[t]:so[t] + KT],
                                    in_=prod[:].rearrange(
                                        "p (a b) -> p a b", b=32),
                                    axis=mybir.AxisListType.X, op=ALU.add)
                            else:
                                cet = cwp.tile([P, KT], F32, tag="cet")
                                nc.vector.tensor_reduce(
                                    out=cet[:],
                                    in_=prod[:].rearrange(
                                        "p (a b) -> p a b", b=32),
                                    axis=mybir.AxisListType.X, op=ALU.add)
                                nc.vector.tensor_tensor(
                                    out=ce_all[:, so[t]:so[t] + KT],
                                    in0=cet[:],
                                    in1=celo[:, so[t]:so[t] + KT], op=ALU.add)

            # ================= scope 3: phase C =============================
            with tc.tile_pool(name="d_const", bufs=1) as dcp, \
                 tc.tile_pool(name="d_xe", bufs=3) as dxp, \
                 tc.tile_pool(name="d_aux", bufs=2) as dauxp, \
                 tc.tile_pool(name="d_w", bufs=3) as dwp, \
                 tc.tile_pool(name="d_vps", bufs=3, space="PSUM") as dvps, \
                 tc.tile_pool(name="d_tps", bufs=2, space="PSUM") as dtps, \
                 tc.tile_pool(name="d_t1ps", bufs=1, space="PSUM") as dt1ps, \
                 tc.tile_pool(name="d_ops", bufs=1, space="PSUM") as dops:
                wv_sb = dcp.tile([P, P], FP16)
                nc.sync.dma_start(out=wv_sb[:], in_=wv_in[:, :])
                wo_sb = dcp.tile([P, P], FP16)
                nc.sync.dma_start(out=wo_sb[:], in_=wo_in[:, :])
                wpw_sb = dcp.tile([4, P], FP16)
                nc.sync.dma_start(out=wpw_sb[:], in_=wpw_in[:, :])
                ident16 = dcp.tile([P, P], FP16)
                make_identity(nc, ident16[:])
                aux_sb = dcp.tile([P, SUMK * 5], F32)
                nc.sync.dma_start(out=aux_sb[:], in_=aux[:, :])
                if use_vb:
                    vbr_sb = dcp.tile([1, P], FP16)
                    nc.sync.dma_start(out=vbr_sb[:], in_=vbr_in[:, :])
                    ones1f = dcp.tile([1, P], FP16)
                    nc.vector.memset(ones1f[:], 1.0)

                with nc.named_scope("phaseC"):
                    for t in range(TO):
                        KT = kts[t]
                        xe_t = dxp.tile([P, KT * P], FP16, tag="xe")
                        nc.sync.dma_start(
                            out=xe_t[:], in_=xeT[:, so[t] * P:(so[t] + KT) * P])
                        xo_t = dauxp.tile([P, P], F32, tag="xo")
                        nc.sync.dma_start(
                            out=xo_t[:], in_=xT_own[:, t * P:(t + 1) * P])

                        # scores + masked softmax
                        s_t = dwp.tile([P, KT], F32, tag="s")
                        bias_view = bass.AP(aux_sb.tensor,
                                            aux_sb[:].offset + so[t] * 5 + 4,
                                            [aux_sb[:].ap[0], (5, KT)])
                        nc.vector.scalar_tensor_tensor(
                            out=s_t[:], in0=ce_all[:, so[t]:so[t] + KT],
                            scalar=strip[:, t:t + 1],
                            in1=bias_view, op0=ALU.mult, op1=ALU.add)
                        negmax = dwp.tile([P, 1], F32, tag="nm")
                        nc.vector.tensor_reduce(
                            out=negmax[:], in_=s_t[:], axis=mybir.AxisListType.X,
                            op=ALU.max, negate=True)
                        e_t = dwp.tile([P, KT], F32, tag="e")
                        esum = dwp.tile([P, 1], F32, tag="es")
                        nc.scalar.activation(
                            out=e_t[:], in_=s_t[:], func=AF.Exp,
                            bias=negmax[:, 0:1], scale=1.0,
                            accum_out=esum[:, 0:1])
                        rs = dwp.tile([P, 1], F32, tag="rsx")
                        nc.vector.reciprocal(out=rs[:], in_=esum[:])
                        w_t = dwp.tile([P, KT], F32, tag="w")
                        nc.vector.tensor_scalar_mul(out=w_t[:], in0=e_t[:],
                                                    scalar1=rs[:, 0:1])

                        # pos: aggregate coords4 with attn weights
                        c4_view = bass.AP(aux_sb.tensor,
                                          aux_sb[:].offset + so[t] * 5,
                                          [aux_sb[:].ap[0], (5, KT), (1, 4)])
                        w_bc = bass.AP(w_t.tensor, w_t[:].offset,
                                       [w_t[:].ap[0], (1, KT), (0, 4)])
                        tmp4 = dwp.tile([P, KT * 4], F32, tag="t4")
                        nc.vector.tensor_tensor(
                            out=tmp4[:].rearrange("p (a b) -> p a b", b=4),
                            in0=c4_view, in1=w_bc, op=ALU.mult)
                        ag4 = dwp.tile([P, 4], F32, tag="a4")
                        ag4_in = bass.AP(tmp4.tensor, tmp4[:].offset,
                                         [tmp4[:].ap[0], (1, 4), (4, KT)])
                        nc.vector.tensor_reduce(
                            out=ag4[:], in_=ag4_in, axis=mybir.AxisListType.X,
                            op=ALU.add)
                        ag416 = dwp.tile([P, 4], FP16, tag="a416")
                        nc.scalar.copy(out=ag416[:], in_=ag4[:])
                        a4T_ps = dt1ps.tile([4, P], FP16, tag="a4T",
                                            padded_shape=[P, P])
                        nc.tensor.transpose(out=a4T_ps[:], in_=ag416[:],
                                            identity=ident16[:])
                        a4T = dwp.tile([4, P], FP16, tag="a4Ts")
                        nc.scalar.copy(out=a4T[:], in_=a4T_ps[:])

                        # weighted aggregation of v (points on out partitions)
                        accA = dwp.tile([P, P], FP16, tag="accA")
                        accB = dwp.tile([P, P], FP16, tag="accB")
                        for k0 in range(0, KT, 4):
                            nk = min(4, KT - k0)
                            v4 = dvps.tile([P, 4 * P], F32, tag="v")
                            for j in range(nk):
                                if use_vb:
                                    nc.tensor.matmul(
                                        out=v4[:, j * P:(j + 1) * P],
                                        lhsT=xe_t[:, (k0 + j) * P:
                                                  (k0 + j + 1) * P],
                                        rhs=wv_sb[:], start=True, stop=False)
                                    nc.tensor.matmul(
                                        out=v4[:, j * P:(j + 1) * P],
                                        lhsT=ones1f[:], rhs=vbr_sb[:],
                                        start=False, stop=True)
                                else:
                                    nc.tensor.matmul(
                                        out=v4[:, j * P:(j + 1) * P],
                                        lhsT=xe_t[:, (k0 + j) * P:
                                                  (k0 + j + 1) * P],
                                        rhs=wv_sb[:], start=True, stop=True)
                            vT4 = dwp.tile([P, 4 * P], FP16, tag="vT")
                            if (k0 // 4) % 2 == 0:
                                nc.scalar.activation(
                                    out=vT4[:, 0:nk * P], in_=v4[:, 0:nk * P],
                                    func=AF.Relu)
                            else:
                                nc.vector.tensor_scalar_max(
                                    out=vT4[:, 0:nk * P], in0=v4[:, 0:nk * P],
                                    scalar1=0.0)
                            for j in range(nk):
                                k = k0 + j
                                sl = vT4[:, j * P:(j + 1) * P]
                                wk = w_t[:, k:k + 1]
                                if k == 0:
                                    nc.vector.tensor_scalar_mul(
                                        out=accA[:], in0=sl, scalar1=wk)
                                elif k == 1:
                                    nc.vector.tensor_scalar_mul(
                                        out=accB[:], in0=sl, scalar1=wk)
                                elif k % 2 == 0:
                                    nc.vector.scalar_tensor_tensor(
                                        out=accA[:], in0=sl, scalar=wk,
                                        op0=ALU.mult, in1=accA[:], op1=ALU.add)
                                else:
                                    nc.vector.scalar_tensor_tensor(
                                        out=accB[:], in0=sl, scalar=wk,
                                        op0=ALU.mult, in1=accB[:], op1=ALU.add)
                        acc = dwp.tile([P, P], FP16, tag="acc")
                        if KT == 1:
                            nc.vector.tensor_copy(out=acc[:], in_=accA[:])
                        else:
                            nc.vector.tensor_tensor(
                                out=acc[:], in0=accA[:], in1=accB[:],
                                op=ALU.add)

                        accT_ps = dt1ps.tile([P, P], FP16, tag="accT")
                        nc.tensor.transpose(out=accT_ps[:], in_=acc[:],
                                            identity=ident16[:])
                        accT = dwp.tile([P, P], FP16, tag="accTs")
                        nc.scalar.copy(out=accT[:], in_=accT_ps[:])
                        o_ps = dops.tile([P, P], F32, tag="o")
                        nc.tensor.matmul(out=o_ps[:], lhsT=wo_sb[:], rhs=accT[:],
                                         start=True, stop=False)
                        nc.tensor.matmul(out=o_ps[:], lhsT=wpw_sb[:], rhs=a4T[:],
                                         start=False, stop=True)
                        oT = dwp.tile([P, P], F32, tag="oT")
                        nc.scalar.activation(
                            out=oT[:], in_=o_ps[:], func=AF.Relu,
                            bias=obeta_sb[:, 0:1])
                        res = dwp.tile([P, P], F32, tag="res")
                        nc.vector.tensor_tensor(out=res[:], in0=oT[:],
                                                in1=xo_t[:], op=ALU.add)
                        nc.sync.dma_start(out=outT[:, t * P:(t + 1) * P],
                                          in_=res[:])

    nc.finalize()
    return nc


def _prep(inputs):
    x = np.asarray(inputs["x"], np.float32)
    coords = np.asarray(inputs["coords"], np.float32)
    W_q = np.asarray(inputs["W_q"], np.float32)
    q_gamma = np.asarray(inputs["q_gamma"], np.float32)
    q_beta = np.asarray(inputs["q_beta"], np.float32)
    W_v = np.asarray(inputs["W_v"], np.float32)
    v_gamma = np.asarray(inputs["v_gamma"], np.float32)
    v_beta = np.asarray(inputs["v_beta"], np.float32)
    codebook = np.asarray(inputs["codebook"], np.float32)
    W_choice = np.asarray(inputs["W_choice"], np.float32)
    b_choice = np.asarray(inputs["b_choice"], np.float32)
    W_pos = np.asarray(inputs["W_pos"], np.float32)
    b_pos = np.asarray(inputs["b_pos"], np.float32)
    W_out = np.asarray(inputs["W_out"], np.float32)
    out_gamma = np.asarray(inputs["out_gamma"], np.float32)
    out_beta = np.asarray(inputs["out_beta"], np.float32)
    nbr_idx = np.asarray(inputs["nbr_idx"], np.int32)
    nbr_mask = np.asarray(inputs["nbr_mask"], np.int32)

    n = x.shape[0]
    assert n == N

    # ---- valid-degree sort (per core shard) -> global relabeling ----
    mask_pad = np.zeros((K, NTOT), bool)
    mask_pad[:, :n] = nbr_mask > 0
    deg = mask_pad.sum(0)
    orders = []
    degs_sorted = np.empty((NCORE, NSH), np.int64)
    for r in range(NCORE):
        sl = slice(r * NSH, (r + 1) * NSH)
        o = np.argsort(-deg[sl], kind="stable")
        orders.append(o)
        degs_sorted[r] = deg[sl][o]
    kts = tuple(int(max(1, degs_sorted[:, t * P:(t + 1) * P].max()))
                for t in range(TO))
    SUMK = sum(kts)
    perm_full = np.concatenate([r * NSH + orders[r] for r in range(NCORE)])
    inv = np.empty(NTOT, np.int64)
    inv[perm_full] = np.arange(NTOT)

    # ---- permuted global tables (new-id order) ----
    xp = np.zeros((NTOT, P), np.float32)
    xp[:n] = x
    xp2 = xp[perm_full]
    x16g = xp2.astype(np.float16)
    cp = np.zeros((NTOT, 3), np.float32)
    cp[:n] = coords
    c4g = np.ones((NTOT, 4), np.float32)
    c4g[:, :3] = cp[perm_full]

    # ---- weight folds ----
    cb2 = float(np.dot(codebook, codebook))
    scb = np.sqrt(cb2).astype(np.float32)
    wcp = codebook[:, None] * W_choice
    wcc = scb * wcp.reshape(VEC, P // VEC, P).sum(1)
    bch = (scb * b_choice)[None, :]
    use_bch = bool(np.any(b_choice != 0))
    wq_flat = np.ascontiguousarray(
        W_q.transpose(1, 0, 2).reshape(P, K * VEC)).astype(np.float16)
    wv16 = (W_v * v_gamma[None, :]).astype(np.float16)
    use_vb = bool(np.any(v_beta != 0))
    wo = W_out * out_gamma[None, :]
    wo16 = wo.astype(np.float16)
    woB = wo.reshape(VEC, P // VEC, P).sum(1)          # [16, 128]
    wpos4 = np.concatenate([W_pos, b_pos[None, :]], axis=0)  # [4, 16]
    wpw16 = (wpos4 @ woB).astype(np.float16)           # [4, 128]
    rmio = np.tile(np.arange(32, dtype=np.float16)[None, :], (P, 1))

    # ---- per-slot neighbor ids (new ids, valid-first compaction) ----
    idx_new = np.full((K, NTOT), Z, np.int32)
    idx_new[:, :n] = np.where(nbr_mask > 0, inv[nbr_idx], Z).astype(np.int32)
    bias_pad = np.full((K, NTOT), np.float32(NEG), np.float32)
    bias_pad[:, :n] = np.where(nbr_mask > 0, 0.0, NEG).astype(np.float32)
    idx_km = idx_new[:, perm_full]          # k-major (original offsets)
    korder = np.argsort(~mask_pad, axis=0, kind="stable")   # valid ks first
    idx_new = np.take_along_axis(idx_new, korder, axis=0)
    bias_pad = np.take_along_axis(bias_pad, korder, axis=0)
    # permute slot-grid columns to sorted point order
    idx_new = idx_new[:, perm_full]
    bias_pad = bias_pad[:, perm_full]

    shared = dict(w_q=wq_flat, wcc=wcc, bch=bch, wv=wv16, wo=wo16,
                  wpw=wpw16, qg=q_gamma[:, None], qb=q_beta[:, None],
                  vbeta=v_beta[:, None], obeta=out_beta[:, None], rmio=rmio)
    if use_vb:
        shared["vbr"] = v_beta[None, :].astype(np.float16)

    prow = np.arange(P, dtype=np.int64)
    in_maps = []
    for r in range(NCORE):
        sl = slice(r * NSH, (r + 1) * NSH)
        slots = idx_new[:, sl]      # [K, NSH] new ids (compacted)
        biasr = bias_pad[:, sl]     # [K, NSH]
        # k-major edge-expanded x for phase A: [128, TO*K*128]
        ja = idx_km[:, sl]          # [K, NSH]
        jlA = ja.reshape(K, TO, P).transpose(1, 0, 2).ravel()  # (t, k, p)
        xeA_r = np.ascontiguousarray(x16g[jlA].T)

        jl_parts = []
        aux_parts = []
        ilo_parts = []
        ihi_parts = []
        code_parts = []
        for t in range(TO):
            KT = kts[t]
            s_tk = slots[:KT, t * P:(t + 1) * P]      # [KT, 128] (k, p)
            b_tk = biasr[:KT, t * P:(t + 1) * P]
            jl_parts.append(s_tk.ravel())             # (k, p) order
            # aux: [128, KT, 5] -> per-partition (k-major) c4 + bias
            a = np.empty((P, KT, 5), np.float32)
            a[:, :, :4] = c4g[s_tk.T]                 # [128, KT, 4]
            a[:, :, 4] = b_tk.T
            aux_parts.append(a.reshape(P, KT * 5))
            # ce lookup tables
            nn = s_tk.T.astype(np.int64)              # [128, KT]
            valid = b_tk.T == 0.0
            fpn = (nn % P) * COLS + nn // P
            slab = fpn // HALFV
            w_in = fpn % HALFV
            ent = w_in // 2 + 1
            m = fpn % 2
            ilo = np.where(slab == 0, ent, 0).astype(np.int16)
            ihi = np.where(slab == 1, ent, 0).astype(np.int16)
            code = np.where(valid, (prow[:, None] % 16) * 2 + m,
                            -1).astype(np.float16)
            ilo_parts.append(np.concatenate([ilo, ihi], axis=1))
            code_parts.append(code)

        jl = np.concatenate(jl_parts)                 # [SUMK*128]
        xeT_r = np.ascontiguousarray(x16g[jl].T)      # [128, SUMK*128]
        aux_r = np.ascontiguousarray(np.concatenate(aux_parts, axis=1))
        pki_r = np.ascontiguousarray(np.concatenate(ilo_parts, axis=1))
        pkc_r = np.ascontiguousarray(np.concatenate(code_parts, axis=1))

        m = dict(shared)
        m["xeA"] = xeA_r
        m["xeT"] = xeT_r
        m["aux"] = aux_r
        m["pki"] = pki_r
        m["pkc"] = pkc_r
        m["xT_own"] = np.ascontiguousarray(xp2[sl].T)
        in_maps.append(m)
    return in_maps, kts, orders, use_bch, use_vb


def prepare(inputs):
    in_maps, kts, orders, use_bch, use_vb = _prep(inputs)
    key = (kts, use_bch, use_vb)
    if _CACHE.get("key") != key:
        _CACHE["nc"] = _build_nc(kts, use_bch, use_vb)
        _CACHE["key"] = key
    return _CACHE["nc"], in_maps, orders


def assemble(results, orders):
    out = np.empty((NCORE * NSH, P), np.float32)
    for r in range(NCORE):
        out[r * NSH + orders[r]] = results[r]["outT"].T
    return np.ascontiguousarray(out[:N])


def kernel(**inputs):
    nc, in_maps, orders = prepare(inputs)
    res = run_bass_kernel_spmd(nc, in_maps, list(range(NCORE)))
    return assemble(res.results, orders)


if __name__ == "__main__":
    rng = np.random.default_rng(0)
    ins = dict(
        x=rng.standard_normal((N, P)).astype(np.float32),
        coords=(rng.random((N, 3)) * 100).astype(np.float32),
        W_q=rng.standard_normal((K, P, VEC)).astype(np.float32) * (P * K) ** -0.5,
        q_gamma=np.ones(VEC, np.float32), q_beta=np.zeros(VEC, np.float32),
        W_v=rng.standard_normal((P, P)).astype(np.float32) * P ** -0.5,
        v_gamma=np.ones(P, np.float32), v_beta=np.zeros(P, np.float32),
        codebook=rng.standard_normal(P).astype(np.float32) * 0.1,
        W_choice=rng.standard_normal((P, P)).astype(np.float32) * P ** -0.5,
        b_choice=np.zeros(P, np.float32),
        W_pos=rng.standard_normal((3, VEC)).astype(np.float32) * 3 ** -0.5,
        b_pos=np.zeros(VEC, np.float32),
        W_out=rng.standard_normal((P, P)).astype(np.float32) * P ** -0.5,
        out_gamma=np.ones(P, np.float32), out_beta=np.zeros(P, np.float32),
        nbr_idx=rng.integers(0, N, (K, N)).astype(np.int32),
        nbr_mask=rng.integers(0, 2, (K, N)).astype(np.int32),
    )
    out = kernel(**ins)
    print("kernel output", out.shape, out.dtype)


# revision 27
# speedup vs baseline: 1.1869x; 1.0041x over previous
"""Trainium2 Bass kernel for nn_DiscreteQKTRBlock (sparse 3x3x3 neighborhood
attention with a discrete codebook).

Strategy (data-parallel over points, 8 cores), v2 "edge-expanded halo":

The discrete-codebook STE path collapses algebraically:
    s[k,i]  = dq[i] . dq[nbr[k,i]] = ||cb||^2 * choice[i] * choice[nbr[k,i]]
so per-offset scores reduce to scalar products of `choice'` = sqrt(cb2)*choice.

Host-side, neighbor indices are fully known, so we pre-expand a "halo" copy of
x per edge slot (xeT, feature-major fp16).  The device then needs NO random
DRAM gathers for x-dependent data:

  A) per consumer tile: q^T = sum_k Wq_k.T @ xe_k  (PSUM accumulation),
     choice' per own point -> strip
  B) AllGather strip (50KB/core); build a per-partition-replicated SBUF table
     of all 100K choice' values (fp16, two 98KB slabs) and resolve per-edge
     neighbor choice via gpsimd ap_gather + diagonal-mask extraction -> ce
  C) per consumer tile: scores = strip*ce + bias, masked softmax; per-slot
     v^T = relu(Wv.T @ xe_k + beta), PE-transpose, weighted DVE accumulation;
     pos is aggregated as sum_k w_k*coords4 and folded through
     (Wpos_exp @ W_out) into the output matmul; relu + residual.

All weight-affine folds are host-side weight-space transforms only.
"""
import sys
sys.path.insert(0, "/opt/trn_rl_repo")
import numpy as np
import ml_dtypes

from concourse import bass, bacc, mybir
import concourse.tile as tile
from concourse.bass_utils import run_bass_kernel_spmd
from concourse.masks import make_identity

F32 = mybir.dt.float32
FP16 = mybir.dt.float16
I16 = mybir.dt.int16
I32 = mybir.dt.int32

N = 100000
P = 128
VEC = 16
K = 27
NEG = -1e9
NCORE = 8
NSH = 12544                 # points per core (98 tiles of 128)
TO = NSH // P               # 98 own tiles
NTOT = NCORE * NSH          # 100352 global (padded) points
Z = N                       # new-id of the guaranteed all-zero pad row
COLS = NCORE * TO           # 784 columns in the wrapped choice layout
HALFV = NTOT // 2           # 50176 choice values per table slab
ENT = HALFV // 2 + 1        # 25089 entries per slab (d=2, incl. zero entry)

_CACHE = {}


def _build_nc(kts, use_bch, use_vb):
    SUMK = sum(kts)
    so = [int(v) for v in np.concatenate([[0], np.cumsum(kts)])]  # slot offsets
    H1 = TO // 2

    nc = bacc.Bacc(num_devices=NCORE, dynamic_dma_scratch_size=16384)

    # ---------------- inputs ----------------
    xeA = nc.declare_dram_parameter("xeA", [P, TO * K * P], FP16, isOutput=False)
    xeT = nc.declare_dram_parameter("xeT", [P, SUMK * P], FP16, isOutput=False)
    aux = nc.declare_dram_parameter("aux", [P, SUMK * 5], F32, isOutput=False)
    pki = nc.declare_dram_parameter("pki", [P, SUMK * 2], I16, isOutput=False)
    pkc = nc.declare_dram_parameter("pkc", [P, SUMK], FP16, isOutput=False)
    xT_own = nc.declare_dram_parameter("xT_own", [P, NSH], F32, isOutput=False)
    w_q = nc.declare_dram_parameter("w_q", [P, K * VEC], FP16, isOutput=False)
    wcc_in = nc.declare_dram_parameter("wcc", [VEC, P], F32, isOutput=False)
    bch_in = nc.declare_dram_parameter("bch", [1, P], F32, isOutput=False)
    wv_in = nc.declare_dram_parameter("wv", [P, P], FP16, isOutput=False)
    wo_in = nc.declare_dram_parameter("wo", [P, P], FP16, isOutput=False)
    wpw_in = nc.declare_dram_parameter("wpw", [4, P], FP16, isOutput=False)
    if use_vb:
        vbr_in = nc.declare_dram_parameter("vbr", [1, P], FP16, isOutput=False)
    qg_in = nc.declare_dram_parameter("qg", [VEC, 1], F32, isOutput=False)
    qb_in = nc.declare_dram_parameter("qb", [VEC, 1], F32, isOutput=False)
    vbeta_in = nc.declare_dram_parameter("vbeta", [P, 1], F32, isOutput=False)
    obeta_in = nc.declare_dram_parameter("obeta", [P, 1], F32, isOutput=False)
    rmio_in = nc.declare_dram_parameter("rmio", [P, 32], FP16, isOutput=False)

    outT = nc.declare_dram_parameter("outT", [P, NSH], F32, isOutput=True)

    AF = mybir.ActivationFunctionType
    ALU = mybir.AluOpType

    with tile.TileContext(nc) as tc:
        with tc.tile_pool(name="persist", bufs=1) as pp, \
             tc.tile_pool(name="dram", bufs=1, space="DRAM") as dpool:
            strip = pp.tile([P, TO], F32)
            qg_sb = pp.tile([VEC, 1], F32)
            nc.sync.dma_start(out=qg_sb[:], in_=qg_in[:, :])
            qb_sb = pp.tile([VEC, 1], F32)
            nc.sync.dma_start(out=qb_sb[:], in_=qb_in[:, :])
            vbeta_sb = pp.tile([P, 1], F32)
            nc.sync.dma_start(out=vbeta_sb[:], in_=vbeta_in[:, :])
            obeta_sb = pp.tile([P, 1], F32)
            nc.sync.dma_start(out=obeta_sb[:], in_=obeta_in[:, :])
            zero_col = pp.tile([P, 1], F32)
            nc.vector.memset(zero_col[:], 0.0)
            ce_all = pp.tile([P, SUMK], FP16)

            c16d = dpool.tile([P, COLS], FP16)
            ced = dpool.tile([P, SUMK], FP16)
            cc_in1 = dpool.tile([P, H1], F32)
            cc_out1 = dpool.tile([NCORE, P, H1], F32, addr_space="Shared")
            cc_in2 = dpool.tile([P, TO - H1], F32)
            cc_out2 = dpool.tile([NCORE, P, TO - H1], F32, addr_space="Shared")

            # ================= scope 1: phase A + allgather =================
            with tc.tile_pool(name="a_const", bufs=1) as acp, \
                 tc.tile_pool(name="a_xe", bufs=3) as axp, \
                 tc.tile_pool(name="a_w", bufs=3) as awp, \
                 tc.tile_pool(name="a_ps", bufs=2, space="PSUM") as apsp, \
                 tc.tile_pool(name="a_ps2", bufs=2, space="PSUM") as apsp2:
                wq_sb = acp.tile([P, K * VEC], FP16)
                nc.sync.dma_start(out=wq_sb[:], in_=w_q[:, :])
                wcc_sb = acp.tile([VEC, P], F32)
                nc.sync.dma_start(out=wcc_sb[:], in_=wcc_in[:, :])
                if use_bch:
                    bch_sb = acp.tile([1, P], F32)
                    nc.sync.dma_start(out=bch_sb[:], in_=bch_in[:, :])
                    ones1 = acp.tile([1, P], F32)
                    nc.vector.memset(ones1[:], 1.0)

                with nc.named_scope("phaseA"):
                    for tg in range(0, TO, 4):
                        nt = min(4, TO - tg)
                        xe4 = axp.tile([P, 4 * K * P], FP16, tag="xe")
                        nc.sync.dma_start(
                            out=xe4[:, 0:nt * K * P],
                            in_=xeA[:, tg * K * P:(tg + nt) * K * P])
                        q4 = apsp.tile([VEC, 4 * P], F32, tag="q",
                                       padded_shape=[P, 4 * P])
                        for k in range(K):
                            rhs = bass.AP(xe4.tensor, xe4[:].offset + k * P,
                                          [xe4[:].ap[0], (K * P, nt), (1, P)])
                            nc.tensor.matmul(
                                out=q4[:, 0:nt * P],
                                lhsT=wq_sb[:, k * VEC:(k + 1) * VEC],
                                rhs=rhs, start=(k == 0), stop=(k == K - 1))
                        qf = awp.tile([VEC, 4 * P], F32, tag="qf")
                        nc.scalar.activation(
                            out=qf[:, 0:nt * P], in_=q4[:, 0:nt * P],
                            func=AF.Relu, bias=qb_sb[:, 0:1],
                            scale=qg_sb[:, 0:1])
                        for j in range(nt):
                            t = tg + j
                            t_ps = apsp2.tile([P, P], F32, tag="t")
                            if use_bch:
                                nc.tensor.matmul(
                                    out=t_ps[:], lhsT=qf[:, j * P:(j + 1) * P],
                                    rhs=wcc_sb[:], start=True, stop=False)
                                nc.tensor.matmul(
                                    out=t_ps[:], lhsT=ones1[:], rhs=bch_sb[:],
                                    start=False, stop=True)
                            else:
                                nc.tensor.matmul(
                                    out=t_ps[:], lhsT=qf[:, j * P:(j + 1) * P],
                                    rhs=wcc_sb[:], start=True, stop=True)
                            scratch = awp.tile([P, P], FP16, tag="scr")
                            nc.scalar.activation(
                                out=scratch[:], in_=t_ps[:], func=AF.Relu,
                                accum_out=strip[:, t:t + 1])

                with nc.named_scope("gather_choice"):
                    nc.sync.dma_start(out=cc_in1[:], in_=strip[:, 0:H1])
                    nc.gpsimd.collective_compute(
                        "AllGather", ALU.bypass,
                        replica_groups=[list(range(NCORE))],
                        ins=[cc_in1.opt()], outs=[cc_out1.opt()])
                    nc.sync.dma_start(out=cc_in2[:], in_=strip[:, H1:TO])
                    nc.gpsimd.collective_compute(
                        "AllGather", ALU.bypass,
                        replica_groups=[list(range(NCORE))],
                        ins=[cc_in2.opt()], outs=[cc_out2.opt()])

            # ================= scope 2a: choice table to DRAM ===============
            with tc.tile_pool(name="b_ch", bufs=1) as bchp:
                with nc.named_scope("chprep"):
                    ch32 = bchp.tile([P, COLS], F32)
                    ca_rt = ch32[:, 0:COLS].rearrange("p (r t) -> p r t", r=NCORE)
                    nc.sync.dma_start(
                        out=ca_rt[:, :, 0:H1],
                        in_=cc_out1[:, :, :].rearrange("r p t -> p r t"))
                    nc.sync.dma_start(
                        out=ca_rt[:, :, H1:TO],
                        in_=cc_out2[:, :, :].rearrange("r p t -> p r t"))
                    ch16 = bchp.tile([P, COLS], FP16)
                    nc.vector.tensor_copy(out=ch16[:], in_=ch32[:])
                    nc.sync.dma_start(out=c16d[:, :], in_=ch16[:])

            # ================= scope 2b: per-edge choice (ce) ===============
            with tc.tile_pool(name="c_fix", bufs=1) as cfp, \
                 tc.tile_pool(name="c_tab", bufs=1) as ctp, \
                 tc.tile_pool(name="c_pk", bufs=4) as cpkp, \
                 tc.tile_pool(name="c_raw", bufs=4) as crawp, \
                 tc.tile_pool(name="c_w", bufs=2) as cwp:
                rm_sb = cfp.tile([P, 32], FP16)
                nc.sync.dma_start(out=rm_sb[:], in_=rmio_in[:, :])
                celo = cfp.tile([P, SUMK], F32)
                pki_sb = cfp.tile([P, SUMK * 2], I16)
                nc.sync.dma_start(out=pki_sb[:], in_=pki[:, :])
                pkc_sb = cfp.tile([P, SUMK], FP16)
                nc.scalar.dma_start(out=pkc_sb[:], in_=pkc[:, :])


                with nc.named_scope("cepass"):
                    for s in range(2):
                        tab = ctp.tile([P, 2 * ENT], FP16, tag="tab")
                        nc.vector.memset(tab[:, 0:2], 0.0)
                        src = bass.AP(c16d.tensor, s * HALFV,
                                      [(0, P), (1, HALFV)])
                        nc.sync.dma_start(out=tab[:, 2:2 + HALFV], in_=src)
                        for t in range(TO):
                            KT = kts[t]
                            pki_t = cpkp.tile([P, KT], I16, tag="pki")
                            nc.vector.tensor_copy(
                                out=pki_t[:].bitcast(FP16),
                                in_=pki_sb[:, so[t] * 2 + s * KT:
                                           so[t] * 2 + (s + 1) * KT
                                           ].bitcast(FP16))
                            code_t = cpkp.tile([P, KT], FP16, tag="pkc")
                            nc.scalar.copy(
                                out=code_t[:],
                                in_=pkc_sb[:, so[t]:so[t] + KT])
                            raw = crawp.tile([P, 16 * KT * 2], FP16, tag="raw")
                            nc.gpsimd.ap_gather(
                                out_ap=raw[:].rearrange("p (n d) -> p n d", d=2),
                                in_ap=tab[:].rearrange("p (n d) -> p n d", d=2),
                                idxs_ap=pki_t[:, 0:KT],
                                channels=P, num_elems=ENT, d=2,
                                num_idxs=16 * KT)
                            mask = cwp.tile([P, KT * 32], FP16, tag="mk")
                            code_bc = bass.AP(code_t.tensor, code_t[:].offset,
                                              [code_t[:].ap[0], (1, KT),
                                               (0, 32)])
                            rm_bc = bass.AP(rm_sb.tensor, rm_sb[:].offset,
                                            [rm_sb[:].ap[0], (0, KT), (1, 32)])
                            nc.vector.tensor_tensor(
                                out=mask[:].rearrange("p (a b) -> p a b", b=32),
                                in0=code_bc, in1=rm_bc, op=ALU.is_equal)
                            prod = cwp.tile([P, KT * 32], FP16, tag="pr")
                            nc.vector.tensor_tensor(
                                out=prod[:], in0=raw[:], in1=mask[:],
                                op=ALU.mult)
                            if s == 0:
                                nc.vector.tensor_reduce(
                                    out=celo[:, so[t]:so[t] + KT],
                                    in_=prod[:].rearrange(
                                        "p (a b) -> p a b", b=32),
                                    axis=mybir.AxisListType.X, op=ALU.add)
                            else:
                                cet = cwp.tile([P, KT], F32, tag="cet")
                                nc.vector.tensor_reduce(
                                    out=cet[:],
                                    in_=prod[:].rearrange(
                                        "p (a b) -> p a b", b=32),
                                    axis=mybir.AxisListType.X, op=ALU.add)
                                nc.vector.tensor_tensor(
                                    out=ce_all[:, so[t]:so[t] + KT],
                                    in0=cet[:],
                                    in1=celo[:, so[t]:so[t] + KT], op=ALU.add)

            # ================= scope 3: phase C =============================
            with tc.tile_pool(name="d_const", bufs=1) as dcp, \
                 tc.tile_pool(name="d_xe", bufs=3) as dxp, \
                 tc.tile_pool(name="d_aux", bufs=2) as dauxp, \
                 tc.tile_pool(name="d_w", bufs=3) as dwp, \
                 tc.tile_pool(name="d_vps", bufs=3, space="PSUM") as dvps, \
                 tc.tile_pool(name="d_tps", bufs=2, space="PSUM") as dtps, \
                 tc.tile_pool(name="d_t1ps", bufs=1, space="PSUM") as dt1ps, \
                 tc.tile_pool(name="d_ops", bufs=1, space="PSUM") as dops:
                wv_sb = dcp.tile([P, P], FP16)
                nc.sync.dma_start(out=wv_sb[:], in_=wv_in[:, :])
                wo_sb = dcp.tile([P, P], FP16)
                nc.sync.dma_start(out=wo_sb[:], in_=wo_in[:, :])
                wpw_sb = dcp.tile([4, P], FP16)
                nc.sync.dma_start(out=wpw_sb[:], in_=wpw_in[:, :])
                ident16 = dcp.tile([P, P], FP16)
                make_identity(nc, ident16[:])
                aux_sb = dcp.tile([P, SUMK * 5], F32)
                nc.sync.dma_start(out=aux_sb[:], in_=aux[:, :])
                if use_vb:
                    vbr_sb = dcp.tile([1, P], FP16)
                    nc.sync.dma_start(out=vbr_sb[:], in_=vbr_in[:, :])
                    ones1f = dcp.tile([1, P], FP16)
                    nc.vector.memset(ones1f[:], 1.0)

                with nc.named_scope("phaseC"):
                    for t in range(TO):
                        KT = kts[t]
                        xe_t = dxp.tile([P, KT * P], FP16, tag="xe")
                        nc.sync.dma_start(
                            out=xe_t[:], in_=xeT[:, so[t] * P:(so[t] + KT) * P])
                        xo_t = dauxp.tile([P, P], F32, tag="xo")
                        nc.sync.dma_start(
                            out=xo_t[:], in_=xT_own[:, t * P:(t + 1) * P])

                        # scores + masked softmax
                        s_t = dwp.tile([P, KT], F32, tag="s")
                        bias_view = bass.AP(aux_sb.tensor,
                                            aux_sb[:].offset + so[t] * 5 + 4,
                                            [aux_sb[:].ap[0], (5, KT)])
                        nc.vector.scalar_tensor_tensor(
                            out=s_t[:], in0=ce_all[:, so[t]:so[t] + KT],
                            scalar=strip[:, t:t + 1],
                            in1=bias_view, op0=ALU.mult, op1=ALU.add)
                        negmax = dwp.tile([P, 1], F32, tag="nm")
                        nc.vector.tensor_reduce(
                            out=negmax[:], in_=s_t[:], axis=mybir.AxisListType.X,
                            op=ALU.max, negate=True)
                        e_t = dwp.tile([P, KT], F32, tag="e")
                        esum = dwp.tile([P, 1], F32, tag="es")
                        nc.scalar.activation(
                            out=e_t[:], in_=s_t[:], func=AF.Exp,
                            bias=negmax[:, 0:1], scale=1.0,
                            accum_out=esum[:, 0:1])
                        rs = dwp.tile([P, 1], F32, tag="rsx")
                        nc.vector.reciprocal(out=rs[:], in_=esum[:])
                        w_t = dwp.tile([P, KT], F32, tag="w")
                        nc.vector.tensor_scalar_mul(out=w_t[:], in0=e_t[:],
                                                    scalar1=rs[:, 0:1])

                        # pos: aggregate coords4 with attn weights
                        c4_view = bass.AP(aux_sb.tensor,
                                          aux_sb[:].offset + so[t] * 5,
                                          [aux_sb[:].ap[0], (5, KT), (1, 4)])
                        w_bc = bass.AP(w_t.tensor, w_t[:].offset,
                                       [w_t[:].ap[0], (1, KT), (0, 4)])
                        tmp4 = dwp.tile([P, KT * 4], F32, tag="t4")
                        nc.vector.tensor_tensor(
                            out=tmp4[:].rearrange("p (a b) -> p a b", b=4),
                            in0=c4_view, in1=w_bc, op=ALU.mult)
                        ag4 = dwp.tile([P, 4], F32, tag="a4")
                        ag4_in = bass.AP(tmp4.tensor, tmp4[:].offset,
                                         [tmp4[:].ap[0], (1, 4), (4, KT)])
                        nc.vector.tensor_reduce(
                            out=ag4[:], in_=ag4_in, axis=mybir.AxisListType.X,
                            op=ALU.add)
                        ag416 = dwp.tile([P, 4], FP16, tag="a416")
                        nc.scalar.copy(out=ag416[:], in_=ag4[:])
                        a4T_ps = dt1ps.tile([4, P], FP16, tag="a4T",
                                            padded_shape=[P, P])
                        nc.tensor.transpose(out=a4T_ps[:], in_=ag416[:],
                                            identity=ident16[:])
                        a4T = dwp.tile([4, P], FP16, tag="a4Ts")
                        nc.scalar.copy(out=a4T[:], in_=a4T_ps[:])

                        # weighted aggregation of v (points on out partitions)
                        accA = dwp.tile([P, P], FP16, tag="accA")
                        accB = dwp.tile([P, P], FP16, tag="accB")
                        for k0 in range(0, KT, 4):
                            nk = min(4, KT - k0)
                            v4 = dvps.tile([P, 4 * P], F32, tag="v")
                            for j in range(nk):
                                if use_vb:
                                    nc.tensor.matmul(
                                        out=v4[:, j * P:(j + 1) * P],
                                        lhsT=xe_t[:, (k0 + j) * P:
                                                  (k0 + j + 1) * P],
                                        rhs=wv_sb[:], start=True, stop=False)
                                    nc.tensor.matmul(
                                        out=v4[:, j * P:(j + 1) * P],
                                        lhsT=ones1f[:], rhs=vbr_sb[:],
                                        start=False, stop=True)
                                else:
                                    nc.tensor.matmul(
                                        out=v4[:, j * P:(j + 1) * P],
                                        lhsT=xe_t[:, (k0 + j) * P:
                                                  (k0 + j + 1) * P],
                                        rhs=wv_sb[:], start=True, stop=True)
                            vT4 = dwp.tile([P, 4 * P], FP16, tag="vT")
                            if (k0 // 4) % 2 == 0:
                                nc.scalar.activation(
                                    out=vT4[:, 0:nk * P], in_=v4[:, 0:nk * P],
                                    func=AF.Relu)
                            else:
                                nc.vector.tensor_scalar_max(
                                    out=vT4[:, 0:nk * P], in0=v4[:, 0:nk * P],
                                    scalar1=0.0)
                            for j in range(nk):
                                k = k0 + j
                                sl = vT4[:, j * P:(j + 1) * P]
                                wk = w_t[:, k:k + 1]
                                if k == 0:
                                    nc.vector.tensor_scalar_mul(
                                        out=accA[:], in0=sl, scalar1=wk)
                                elif k == 1:
                                    nc.vector.tensor_scalar_mul(
                                        out=accB[:], in0=sl, scalar1=wk)
                                elif k % 2 == 0:
                                    nc.vector.scalar_tensor_tensor(
                                        out=accA[:], in0=sl, scalar=wk,
                                        op0=ALU.mult, in1=accA[:], op1=ALU.add)
                                else:
                                    nc.vector.scalar_tensor_tensor(
                                        out=accB[:], in0=sl, scalar=wk,
                                        op0=ALU.mult, in1=accB[:], op1=ALU.add)
                        acc = dwp.tile([P, P], FP16, tag="acc")
                        if KT == 1:
                            nc.vector.tensor_copy(out=acc[:], in_=accA[:])
                        else:
                            nc.vector.tensor_tensor(
                                out=acc[:], in0=accA[:], in1=accB[:],
                                op=ALU.add)

                        accT_ps = dt1ps.tile([P, P], FP16, tag="accT")
                        nc.tensor.transpose(out=accT_ps[:], in_=acc[:],
                                            identity=ident16[:])
                        accT = dwp.tile([P, P], FP16, tag="accTs")
                        nc.scalar.copy(out=accT[:], in_=accT_ps[:])
                        o_ps = dops.tile([P, P], F32, tag="o")
                        nc.tensor.matmul(out=o_ps[:], lhsT=wo_sb[:], rhs=accT[:],
                                         start=True, stop=False)
                        nc.tensor.matmul(out=o_ps[:], lhsT=wpw_sb[:], rhs=a4T[:],
                                         start=False, stop=True)
                        oT = dwp.tile([P, P], F32, tag="oT")
                        nc.scalar.activation(
                            out=oT[:], in_=o_ps[:], func=AF.Relu,
                            bias=obeta_sb[:, 0:1])
                        res = dwp.tile([P, P], F32, tag="res")
                        nc.vector.tensor_tensor(out=res[:], in0=oT[:],
                                                in1=xo_t[:], op=ALU.add)
                        nc.sync.dma_start(out=outT[:, t * P:(t + 1) * P],
                                          in_=res[:])

    nc.finalize()
    return nc


def _prep(inputs):
    x = np.asarray(inputs["x"], np.float32)
    coords = np.asarray(inputs["coords"], np.float32)
    W_q = np.asarray(inputs["W_q"], np.float32)
    q_gamma = np.asarray(inputs["q_gamma"], np.float32)
    q_beta = np.asarray(inputs["q_beta"], np.float32)
    W_v = np.asarray(inputs["W_v"], np.float32)
    v_gamma = np.asarray(inputs["v_gamma"], np.float32)
    v_beta = np.asarray(inputs["v_beta"], np.float32)
    codebook = np.asarray(inputs["codebook"], np.float32)
    W_choice = np.asarray(inputs["W_choice"], np.float32)
    b_choice = np.asarray(inputs["b_choice"], np.float32)
    W_pos = np.asarray(inputs["W_pos"], np.float32)
    b_pos = np.asarray(inputs["b_pos"], np.float32)
    W_out = np.asarray(inputs["W_out"], np.float32)
    out_gamma = np.asarray(inputs["out_gamma"], np.float32)
    out_beta = np.asarray(inputs["out_beta"], np.float32)
    nbr_idx = np.asarray(inputs["nbr_idx"], np.int32)
    nbr_mask = np.asarray(inputs["nbr_mask"], np.int32)

    n = x.shape[0]
    assert n == N

    # ---- valid-degree sort (per core shard) -> global relabeling ----
    mask_pad = np.zeros((K, NTOT), bool)
    mask_pad[:, :n] = nbr_mask > 0
    deg = mask_pad.sum(0)
    orders = []
    degs_sorted = np.empty((NCORE, NSH), np.int64)
    for r in range(NCORE):
        sl = slice(r * NSH, (r + 1) * NSH)
        o = np.argsort(-deg[sl], kind="stable")
        orders.append(o)
        degs_sorted[r] = deg[sl][o]
    kts = tuple(int(max(1, degs_sorted[:, t * P:(t + 1) * P].max()))
                for t in range(TO))
    SUMK = sum(kts)
    perm_full = np.concatenate([r * NSH + orders[r] for r in range(NCORE)])
    inv = np.empty(NTOT, np.int64)
    inv[perm_full] = np.arange(NTOT)

    # ---- permuted global tables (new-id order) ----
    xp = np.zeros((NTOT, P), np.float32)
    xp[:n] = x
    xp2 = xp[perm_full]
    x16g = xp2.astype(np.float16)
    cp = np.zeros((NTOT, 3), np.float32)
    cp[:n] = coords
    c4g = np.ones((NTOT, 4), np.float32)
    c4g[:, :3] = cp[perm_full]

    # ---- weight folds ----
    cb2 = float(np.dot(codebook, codebook))
    scb = np.sqrt(cb2).astype(np.float32)
    wcp = codebook[:, None] * W_choice
    wcc = scb * wcp.reshape(VEC, P // VEC, P).sum(1)
    bch = (scb * b_choice)[None, :]
    use_bch = bool(np.any(b_choice != 0))
    wq_flat = np.ascontiguousarray(
        W_q.transpose(1, 0, 2).reshape(P, K * VEC)).astype(np.float16)
    wv16 = (W_v * v_gamma[None, :]).astype(np.float16)
    use_vb = bool(np.any(v_beta != 0))
    wo = W_out * out_gamma[None, :]
    wo16 = wo.astype(np.float16)
    woB = wo.reshape(VEC, P // VEC, P).sum(1)          # [16, 128]
    wpos4 = np.concatenate([W_pos, b_pos[None, :]], axis=0)  # [4, 16]
    wpw16 = (wpos4 @ woB).astype(np.float16)           # [4, 128]
    rmio = np.tile(np.arange(32, dtype=np.float16)[None, :], (P, 1))

    # ---- per-slot neighbor ids (new ids, valid-first compaction) ----
    idx_new = np.full((K, NTOT), Z, np.int32)
    idx_new[:, :n] = np.where(nbr_mask > 0, inv[nbr_idx], Z).astype(np.int32)
    bias_pad = np.full((K, NTOT), np.float32(NEG), np.float32)
    bias_pad[:, :n] = np.where(nbr_mask > 0, 0.0, NEG).astype(np.float32)
    idx_km = idx_new[:, perm_full]          # k-major (original offsets)
    korder = np.argsort(~mask_pad, axis=0, kind="stable")   # valid ks first
    idx_new = np.take_along_axis(idx_new, korder, axis=0)
    bias_pad = np.take_along_axis(bias_pad, korder, axis=0)
    # permute slot-grid columns to sorted point order
    idx_new = idx_new[:, perm_full]
    bias_pad = bias_pad[:, perm_full]

    shared = dict(w_q=wq_flat, wcc=wcc, bch=bch, wv=wv16, wo=wo16,
                  wpw=wpw16, qg=q_gamma[:, None], qb=q_beta[:, None],
                  vbeta=v_beta[:, None], obeta=out_beta[:, None], rmio=rmio)
    if use_vb:
        shared["vbr"] = v_beta[None, :].astype(np.float16)

    prow = np.arange(P, dtype=np.int64)
    in_maps = []
    for r in range(NCORE):
        sl = slice(r * NSH, (r + 1) * NSH)
        slots = idx_new[:, sl]      # [K, NSH] new ids (compacted)
        biasr = bias_pad[:, sl]     # [K, NSH]
        # k-major edge-expanded x for phase A: [128, TO*K*128]
        ja = idx_km[:, sl]          # [K, NSH]
        jlA = ja.reshape(K, TO, P).transpose(1, 0, 2).ravel()  # (t, k, p)
        xeA_r = np.ascontiguousarray(x16g[jlA].T)

        jl_parts = []
        aux_parts = []
        ilo_parts = []
        ihi_parts = []
        code_parts = []
        for t in range(TO):
            KT = kts[t]
            s_tk = slots[:KT, t * P:(t + 1) * P]      # [KT, 128] (k, p)
            b_tk = biasr[:KT, t * P:(t + 1) * P]
            jl_parts.append(s_tk.ravel())             # (k, p) order
            # aux: [128, KT, 5] -> per-partition (k-major) c4 + bias
            a = np.empty((P, KT, 5), np.float32)
            a[:, :, :4] = c4g[s_tk.T]                 # [128, KT, 4]
            a[:, :, 4] = b_tk.T
            aux_parts.append(a.reshape(P, KT * 5))
            # ce lookup tables
            nn = s_tk.T.astype(np.int64)              # [128, KT]
            valid = b_tk.T == 0.0
            fpn = (nn % P) * COLS + nn // P
            slab = fpn // HALFV
            w_in = fpn % HALFV
            ent = w_in // 2 + 1
            m = fpn % 2
            ilo = np.where(slab == 0, ent, 0).astype(np.int16)
            ihi = np.where(slab == 1, ent, 0).astype(np.int16)
            code = np.where(valid, (prow[:, None] % 16) * 2 + m,
                            -1).astype(np.float16)
            ilo_parts.append(np.concatenate([ilo, ihi], axis=1))
            code_parts.append(code)

        jl = np.concatenate(jl_parts)                 # [SUMK*128]
        xeT_r = np.ascontiguousarray(x16g[jl].T)      # [128, SUMK*128]
        aux_r = np.ascontiguousarray(np.concatenate(aux_parts, axis=1))
        pki_r = np.ascontiguousarray(np.concatenate(ilo_parts, axis=1))
        pkc_r = np.ascontiguousarray(np.concatenate(code_parts, axis=1))

        m = dict(shared)
        m["xeA"] = xeA_r
        m["xeT"] = xeT_r
        m["aux"] = aux_r
        m["pki"] = pki_r
        m["pkc"] = pkc_r
        m["xT_own"] = np.ascontiguousarray(xp2[sl].T)
        in_maps.append(m)
    return in_maps, kts, orders, use_bch, use_vb


def prepare(inputs):
    in_maps, kts, orders, use_bch, use_vb = _prep(inputs)
    key = (kts, use_bch, use_vb)
    if _CACHE.get("key") != key:
        _CACHE["nc"] = _build_nc(kts, use_bch, use_vb)
        _CACHE["key"] = key
    return _CACHE["nc"], in_maps, orders


def assemble(results, orders):
    out = np.empty((NCORE * NSH, P), np.float32)
    for r in range(NCORE):
        out[r * NSH + orders[r]] = results[r]["outT"].T
    return np.ascontiguousarray(out[:N])


def kernel(**inputs):
    nc, in_maps, orders = prepare(inputs)
    res = run_bass_kernel_spmd(nc, in_maps, list(range(NCORE)))
    return assemble(res.results, orders)


if __name__ == "__main__":
    rng = np.random.default_rng(0)
    ins = dict(
        x=rng.standard_normal((N, P)).astype(np.float32),
        coords=(rng.random((N, 3)) * 100).astype(np.float32),
        W_q=rng.standard_normal((K, P, VEC)).astype(np.float32) * (P * K) ** -0.5,
        q_gamma=np.ones(VEC, np.float32), q_beta=np.zeros(VEC, np.float32),
        W_v=rng.standard_normal((P, P)).astype(np.float32) * P ** -0.5,
        v_gamma=np.ones(P, np.float32), v_beta=np.zeros(P, np.float32),
        codebook=rng.standard_normal(P).astype(np.float32) * 0.1,
        W_choice=rng.standard_normal((P, P)).astype(np.float32) * P ** -0.5,
        b_choice=np.zeros(P, np.float32),
        W_pos=rng.standard_normal((3, VEC)).astype(np.float32) * 3 ** -0.5,
        b_pos=np.zeros(VEC, np.float32),
        W_out=rng.standard_normal((P, P)).astype(np.float32) * P ** -0.5,
        out_gamma=np.ones(P, np.float32), out_beta=np.zeros(P, np.float32),
        nbr_idx=rng.integers(0, N, (K, N)).astype(np.int32),
        nbr_mask=rng.integers(0, 2, (K, N)).astype(np.int32),
    )
    out = kernel(**ins)
    print("kernel output", out.shape, out.dtype)


# revision 29
# speedup vs baseline: 1.2883x; 1.0855x over previous
"""Trainium2 Bass kernel for nn_DiscreteQKTRBlock (sparse 3x3x3 neighborhood
attention with a discrete codebook).

Strategy (data-parallel over points, 8 cores), v2 "edge-expanded halo":

The discrete-codebook STE path collapses algebraically:
    s[k,i]  = dq[i] . dq[nbr[k,i]] = ||cb||^2 * choice[i] * choice[nbr[k,i]]
so per-offset scores reduce to scalar products of `choice'` = sqrt(cb2)*choice.

Host-side, neighbor indices are fully known, so we pre-expand a "halo" copy of
x per edge slot (xeT, feature-major fp16).  The device then needs NO random
DRAM gathers for x-dependent data:

  A) per consumer tile: q^T = sum_k Wq_k.T @ xe_k  (PSUM accumulation),
     choice' per own point -> strip
  B) AllGather strip (50KB/core); build a per-partition-replicated SBUF table
     of all 100K choice' values (fp16, two 98KB slabs) and resolve per-edge
     neighbor choice via gpsimd ap_gather + diagonal-mask extraction -> ce
  C) per consumer tile: scores = strip*ce + bias, masked softmax; per-slot
     v^T = relu(Wv.T @ xe_k + beta), PE-transpose, weighted DVE accumulation;
     pos is aggregated as sum_k w_k*coords4 and folded through
     (Wpos_exp @ W_out) into the output matmul; relu + residual.

All weight-affine folds are host-side weight-space transforms only.
"""
import sys
sys.path.insert(0, "/opt/trn_rl_repo")
import numpy as np
import ml_dtypes

from concourse import bass, bacc, mybir
import concourse.tile as tile
from concourse.bass_utils import run_bass_kernel_spmd
from concourse.masks import make_identity

F32 = mybir.dt.float32
FP16 = mybir.dt.float16
I16 = mybir.dt.int16
I32 = mybir.dt.int32

N = 100000
P = 128
VEC = 16
K = 27
NEG = -1e9
NCORE = 8
NSH = 12544                 # points per core (98 tiles of 128)
TO = NSH // P               # 98 own tiles
NTOT = NCORE * NSH          # 100352 global (padded) points
Z = N                       # new-id of the guaranteed all-zero pad row
COLS = NCORE * TO           # 784 columns in the wrapped choice layout
HALFV = NTOT // 2           # 50176 choice values per table slab
ENT = HALFV // 2 + 1        # 25089 entries per slab (d=2, incl. zero entry)

_CACHE = {}


def _build_nc(kts, use_bch, use_vb):
    SUMK = sum(kts)
    so = [int(v) for v in np.concatenate([[0], np.cumsum(kts)])]  # slot offsets
    H1 = TO // 2

    nc = bacc.Bacc(num_devices=NCORE, dynamic_dma_scratch_size=16384)

    # ---------------- inputs ----------------
    xeA = nc.declare_dram_parameter("xeA", [P, TO * K * P], FP16, isOutput=False)
    xeT = nc.declare_dram_parameter("xeT", [P, SUMK * P], FP16, isOutput=False)
    aux = nc.declare_dram_parameter("aux", [P, SUMK * 5], F32, isOutput=False)
    pki = nc.declare_dram_parameter("pki", [P, SUMK * 2], I16, isOutput=False)
    pkc = nc.declare_dram_parameter("pkc", [P, SUMK], FP16, isOutput=False)
    xT_own = nc.declare_dram_parameter("xT_own", [P, NSH], F32, isOutput=False)
    w_q = nc.declare_dram_parameter("w_q", [P, K * VEC], FP16, isOutput=False)
    wcc_in = nc.declare_dram_parameter("wcc", [VEC, P], F32, isOutput=False)
    bch_in = nc.declare_dram_parameter("bch", [1, P], F32, isOutput=False)
    wv_in = nc.declare_dram_parameter("wv", [P, P], FP16, isOutput=False)
    wo_in = nc.declare_dram_parameter("wo", [P, P], FP16, isOutput=False)
    wpw_in = nc.declare_dram_parameter("wpw", [4, P], FP16, isOutput=False)
    if use_vb:
        vbr_in = nc.declare_dram_parameter("vbr", [1, P], FP16, isOutput=False)
    qg_in = nc.declare_dram_parameter("qg", [VEC, 1], F32, isOutput=False)
    qb_in = nc.declare_dram_parameter("qb", [VEC, 1], F32, isOutput=False)
    vbeta_in = nc.declare_dram_parameter("vbeta", [P, 1], F32, isOutput=False)
    obeta_in = nc.declare_dram_parameter("obeta", [P, 1], F32, isOutput=False)
    rmio_in = nc.declare_dram_parameter("rmio", [P, 32], FP16, isOutput=False)

    outT = nc.declare_dram_parameter("outT", [P, NSH], F32, isOutput=True)

    AF = mybir.ActivationFunctionType
    ALU = mybir.AluOpType

    with tile.TileContext(nc) as tc:
        with tc.tile_pool(name="persist", bufs=1) as pp, \
             tc.tile_pool(name="dram", bufs=1, space="DRAM") as dpool:
            strip = pp.tile([P, TO], F32)
            qg_sb = pp.tile([VEC, 1], F32)
            nc.sync.dma_start(out=qg_sb[:], in_=qg_in[:, :])
            qb_sb = pp.tile([VEC, 1], F32)
            nc.sync.dma_start(out=qb_sb[:], in_=qb_in[:, :])
            vbeta_sb = pp.tile([P, 1], F32)
            nc.sync.dma_start(out=vbeta_sb[:], in_=vbeta_in[:, :])
            obeta_sb = pp.tile([P, 1], F32)
            nc.sync.dma_start(out=obeta_sb[:], in_=obeta_in[:, :])
            zero_col = pp.tile([P, 1], F32)
            nc.vector.memset(zero_col[:], 0.0)
            ce_all = pp.tile([P, SUMK], FP16)

            c16d = dpool.tile([P, COLS], FP16)
            ced = dpool.tile([P, SUMK], FP16)
            cc_in1 = dpool.tile([P, H1], F32)
            cc_out1 = dpool.tile([NCORE, P, H1], F32, addr_space="Shared")
            cc_in2 = dpool.tile([P, TO - H1], F32)
            cc_out2 = dpool.tile([NCORE, P, TO - H1], F32, addr_space="Shared")

            # ================= scope 1: phase A + allgather =================
            with tc.tile_pool(name="a_const", bufs=1) as acp, \
                 tc.tile_pool(name="a_xe", bufs=3) as axp, \
                 tc.tile_pool(name="a_w", bufs=3) as awp, \
                 tc.tile_pool(name="a_ps", bufs=2, space="PSUM") as apsp, \
                 tc.tile_pool(name="a_ps2", bufs=2, space="PSUM") as apsp2:
                wq_sb = acp.tile([P, K * VEC], FP16)
                nc.sync.dma_start(out=wq_sb[:], in_=w_q[:, :])
                wcc_sb = acp.tile([VEC, P], F32)
                nc.sync.dma_start(out=wcc_sb[:], in_=wcc_in[:, :])
                if use_bch:
                    bch_sb = acp.tile([1, P], F32)
                    nc.sync.dma_start(out=bch_sb[:], in_=bch_in[:, :])
                    ones1 = acp.tile([1, P], F32)
                    nc.vector.memset(ones1[:], 1.0)

                with nc.named_scope("phaseA"):
                    for tg in range(0, TO, 4):
                        nt = min(4, TO - tg)
                        xe4 = axp.tile([P, 4 * K * P], FP16, tag="xe")
                        nc.sync.dma_start(
                            out=xe4[:, 0:nt * K * P],
                            in_=xeA[:, tg * K * P:(tg + nt) * K * P])
                        q4 = apsp.tile([VEC, 4 * P], F32, tag="q",
                                       padded_shape=[P, 4 * P])
                        for k in range(K):
                            rhs = bass.AP(xe4.tensor, xe4[:].offset + k * P,
                                          [xe4[:].ap[0], (K * P, nt), (1, P)])
                            nc.tensor.matmul(
                                out=q4[:, 0:nt * P],
                                lhsT=wq_sb[:, k * VEC:(k + 1) * VEC],
                                rhs=rhs, start=(k == 0), stop=(k == K - 1))
                        qf = awp.tile([VEC, 4 * P], F32, tag="qf")
                        nc.scalar.activation(
                            out=qf[:, 0:nt * P], in_=q4[:, 0:nt * P],
                            func=AF.Relu, bias=qb_sb[:, 0:1],
                            scale=qg_sb[:, 0:1])
                        for j in range(nt):
                            t = tg + j
                            t_ps = apsp2.tile([P, P], F32, tag="t")
                            if use_bch:
                                nc.tensor.matmul(
                                    out=t_ps[:], lhsT=qf[:, j * P:(j + 1) * P],
                                    rhs=wcc_sb[:], start=True, stop=False)
                                nc.tensor.matmul(
                                    out=t_ps[:], lhsT=ones1[:], rhs=bch_sb[:],
                                    start=False, stop=True)
                            else:
                                nc.tensor.matmul(
                                    out=t_ps[:], lhsT=qf[:, j * P:(j + 1) * P],
                                    rhs=wcc_sb[:], start=True, stop=True)
                            scratch = awp.tile([P, P], FP16, tag="scr")
                            nc.scalar.activation(
                                out=scratch[:], in_=t_ps[:], func=AF.Relu,
                                accum_out=strip[:, t:t + 1])

                with nc.named_scope("gather_choice"):
                    nc.sync.dma_start(out=cc_in1[:], in_=strip[:, 0:H1])
                    nc.gpsimd.collective_compute(
                        "AllGather", ALU.bypass,
                        replica_groups=[list(range(NCORE))],
                        ins=[cc_in1.opt()], outs=[cc_out1.opt()])
                    nc.sync.dma_start(out=cc_in2[:], in_=strip[:, H1:TO])
                    nc.gpsimd.collective_compute(
                        "AllGather", ALU.bypass,
                        replica_groups=[list(range(NCORE))],
                        ins=[cc_in2.opt()], outs=[cc_out2.opt()])

            # ================= scope 2a: choice table to DRAM ===============
            with tc.tile_pool(name="b_ch", bufs=1) as bchp:
                with nc.named_scope("chprep"):
                    ch32 = bchp.tile([P, COLS], F32)
                    ca_rt = ch32[:, 0:COLS].rearrange("p (r t) -> p r t", r=NCORE)
                    nc.sync.dma_start(
                        out=ca_rt[:, :, 0:H1],
                        in_=cc_out1[:, :, :].rearrange("r p t -> p r t"))
                    nc.sync.dma_start(
                        out=ca_rt[:, :, H1:TO],
                        in_=cc_out2[:, :, :].rearrange("r p t -> p r t"))
                    ch16 = bchp.tile([P, COLS], FP16)
                    nc.vector.tensor_copy(out=ch16[:], in_=ch32[:])
                    nc.sync.dma_start(out=c16d[:, :], in_=ch16[:])

            # ================= scope 2b: per-edge choice (ce) ===============
            from contextlib import ExitStack
            cstk = ExitStack()
            with tc.tile_pool(name="c_fix", bufs=1) as cfp, \
                 tc.tile_pool(name="c_tab", bufs=1) as ctp, \
                 tc.tile_pool(name="c_pk", bufs=4) as cpkp, \
                 tc.tile_pool(name="c_raw", bufs=4) as crawp, \
                 tc.tile_pool(name="c_w", bufs=2) as cwp:
                rm_sb = cfp.tile([P, 32], FP16)
                nc.sync.dma_start(out=rm_sb[:], in_=rmio_in[:, :])
                celo = cfp.tile([P, SUMK], F32)


                with nc.named_scope("cepass"):
                    for s in range(2):
                        tab = ctp.tile([P, 2 * ENT], FP16, tag="tab")
                        nc.vector.memset(tab[:, 0:2], 0.0)
                        src = bass.AP(c16d.tensor, s * HALFV,
                                      [(0, P), (1, HALFV)])
                        nc.sync.dma_start(out=tab[:, 2:2 + HALFV], in_=src)
                        for t in range(TO):
                            KT = kts[t]
                            pki_t = cpkp.tile([P, KT], I16, tag="pki")
                            nc.sync.dma_start(
                                out=pki_t[:],
                                in_=pki[:, so[t] * 2 + s * KT:
                                        so[t] * 2 + (s + 1) * KT])
                            code_t = cpkp.tile([P, KT], FP16, tag="pkc")
                            nc.scalar.dma_start(
                                out=code_t[:], in_=pkc[:, so[t]:so[t] + KT])
                            raw = crawp.tile([P, 16 * KT * 2], FP16, tag="raw")
                            nc.gpsimd.ap_gather(
                                out_ap=raw[:].rearrange("p (n d) -> p n d", d=2),
                                in_ap=tab[:].rearrange("p (n d) -> p n d", d=2),
                                idxs_ap=pki_t[:, 0:KT],
                                channels=P, num_elems=ENT, d=2,
                                num_idxs=16 * KT)
                            mask = cwp.tile([P, KT * 32], FP16, tag="mk")
                            code_bc = bass.AP(code_t.tensor, code_t[:].offset,
                                              [code_t[:].ap[0], (1, KT),
                                               (0, 32)])
                            rm_bc = bass.AP(rm_sb.tensor, rm_sb[:].offset,
                                            [rm_sb[:].ap[0], (0, KT), (1, 32)])
                            nc.vector.tensor_tensor(
                                out=mask[:].rearrange("p (a b) -> p a b", b=32),
                                in0=code_bc, in1=rm_bc, op=ALU.is_equal)
                            prod = cwp.tile([P, KT * 32], FP16, tag="pr")
                            nc.vector.tensor_tensor(
                                out=prod[:], in0=raw[:], in1=mask[:],
                                op=ALU.mult)
                            if s == 0:
                                nc.vector.tensor_reduce(
                                    out=celo[:, so[t]:so[t] + KT],
                                    in_=prod[:].rearrange(
                                        "p (a b) -> p a b", b=32),
                                    axis=mybir.AxisListType.X, op=ALU.add)
                            else:
                                cet = cwp.tile([P, KT], F32, tag="cet")
                                nc.vector.tensor_reduce(
                                    out=cet[:],
                                    in_=prod[:].rearrange(
                                        "p (a b) -> p a b", b=32),
                                    axis=mybir.AxisListType.X, op=ALU.add)
                                nc.vector.tensor_tensor(
                                    out=ce_all[:, so[t]:so[t] + KT],
                                    in0=cet[:],
                                    in1=celo[:, so[t]:so[t] + KT], op=ALU.add)

                # ---- phase C shares this scope so it overlaps cepass ----
                dcp = cstk.enter_context(tc.tile_pool(name="d_const", bufs=1))
                dxp = cstk.enter_context(tc.tile_pool(name="d_xe", bufs=3))
                dauxp = cstk.enter_context(tc.tile_pool(name="d_aux", bufs=2))
                dwp = cstk.enter_context(tc.tile_pool(name="d_w", bufs=3))
                dvps = cstk.enter_context(
                    tc.tile_pool(name="d_vps", bufs=3, space="PSUM"))
                dt1ps = cstk.enter_context(
                    tc.tile_pool(name="d_t1ps", bufs=1, space="PSUM"))
                dops = cstk.enter_context(
                    tc.tile_pool(name="d_ops", bufs=1, space="PSUM"))
                wv_sb = dcp.tile([P, P], FP16)
                nc.sync.dma_start(out=wv_sb[:], in_=wv_in[:, :])
                wo_sb = dcp.tile([P, P], FP16)
                nc.sync.dma_start(out=wo_sb[:], in_=wo_in[:, :])
                wpw_sb = dcp.tile([4, P], FP16)
                nc.sync.dma_start(out=wpw_sb[:], in_=wpw_in[:, :])
                ident16 = dcp.tile([P, P], FP16)
                make_identity(nc, ident16[:])
                aux_sb = dcp.tile([P, SUMK * 5], F32)
                nc.sync.dma_start(out=aux_sb[:], in_=aux[:, :])
                if use_vb:
                    vbr_sb = dcp.tile([1, P], FP16)
                    nc.sync.dma_start(out=vbr_sb[:], in_=vbr_in[:, :])
                    ones1f = dcp.tile([1, P], FP16)
                    nc.vector.memset(ones1f[:], 1.0)

                with nc.named_scope("phaseC"):
                    for t in range(TO):
                        KT = kts[t]
                        xe_t = dxp.tile([P, KT * P], FP16, tag="xe")
                        nc.sync.dma_start(
                            out=xe_t[:], in_=xeT[:, so[t] * P:(so[t] + KT) * P])
                        xo_t = dauxp.tile([P, P], F32, tag="xo")
                        nc.sync.dma_start(
                            out=xo_t[:], in_=xT_own[:, t * P:(t + 1) * P])

                        # scores + masked softmax
                        s_t = dwp.tile([P, KT], F32, tag="s")
                        bias_view = bass.AP(aux_sb.tensor,
                                            aux_sb[:].offset + so[t] * 5 + 4,
                                            [aux_sb[:].ap[0], (5, KT)])
                        nc.vector.scalar_tensor_tensor(
                            out=s_t[:], in0=ce_all[:, so[t]:so[t] + KT],
                            scalar=strip[:, t:t + 1],
                            in1=bias_view, op0=ALU.mult, op1=ALU.add)
                        negmax = dwp.tile([P, 1], F32, tag="nm")
                        nc.vector.tensor_reduce(
                            out=negmax[:], in_=s_t[:], axis=mybir.AxisListType.X,
                            op=ALU.max, negate=True)
                        e_t = dwp.tile([P, KT], F32, tag="e")
                        esum = dwp.tile([P, 1], F32, tag="es")
                        nc.scalar.activation(
                            out=e_t[:], in_=s_t[:], func=AF.Exp,
                            bias=negmax[:, 0:1], scale=1.0,
                            accum_out=esum[:, 0:1])
                        rs = dwp.tile([P, 1], F32, tag="rsx")
                        nc.vector.reciprocal(out=rs[:], in_=esum[:])
                        w_t = dwp.tile([P, KT], F32, tag="w")
                        nc.vector.tensor_scalar_mul(out=w_t[:], in0=e_t[:],
                                                    scalar1=rs[:, 0:1])

                        # pos: aggregate coords4 with attn weights
                        c4_view = bass.AP(aux_sb.tensor,
                                          aux_sb[:].offset + so[t] * 5,
                                          [aux_sb[:].ap[0], (5, KT), (1, 4)])
                        w_bc = bass.AP(w_t.tensor, w_t[:].offset,
                                       [w_t[:].ap[0], (1, KT), (0, 4)])
                        tmp4 = dwp.tile([P, KT * 4], F32, tag="t4")
                        nc.vector.tensor_tensor(
                            out=tmp4[:].rearrange("p (a b) -> p a b", b=4),
                            in0=c4_view, in1=w_bc, op=ALU.mult)
                        ag4 = dwp.tile([P, 4], F32, tag="a4")
                        ag4_in = bass.AP(tmp4.tensor, tmp4[:].offset,
                                         [tmp4[:].ap[0], (1, 4), (4, KT)])
                        nc.vector.tensor_reduce(
                            out=ag4[:], in_=ag4_in, axis=mybir.AxisListType.X,
                            op=ALU.add)
                        ag416 = dwp.tile([P, 4], FP16, tag="a416")
                        nc.scalar.copy(out=ag416[:], in_=ag4[:])
                        a4T_ps = dt1ps.tile([4, P], FP16, tag="a4T",
                                            padded_shape=[P, P])
                        nc.tensor.transpose(out=a4T_ps[:], in_=ag416[:],
                                            identity=ident16[:])
                        a4T = dwp.tile([4, P], FP16, tag="a4Ts")
                        nc.scalar.copy(out=a4T[:], in_=a4T_ps[:])

                        # weighted aggregation of v (points on out partitions)
                        accA = dwp.tile([P, P], FP16, tag="accA")
                        accB = dwp.tile([P, P], FP16, tag="accB")
                        for k0 in range(0, KT, 4):
                            nk = min(4, KT - k0)
                            v4 = dvps.tile([P, 4 * P], F32, tag="v")
                            for j in range(nk):
                                if use_vb:
                                    nc.tensor.matmul(
                                        out=v4[:, j * P:(j + 1) * P],
                                        lhsT=xe_t[:, (k0 + j) * P:
                                                  (k0 + j + 1) * P],
                                        rhs=wv_sb[:], start=True, stop=False)
                                    nc.tensor.matmul(
                                        out=v4[:, j * P:(j + 1) * P],
                                        lhsT=ones1f[:], rhs=vbr_sb[:],
                                        start=False, stop=True)
                                else:
                                    nc.tensor.matmul(
                                        out=v4[:, j * P:(j + 1) * P],
                                        lhsT=xe_t[:, (k0 + j) * P:
                                                  (k0 + j + 1) * P],
                                        rhs=wv_sb[:], start=True, stop=True)
                            vT4 = dwp.tile([P, 4 * P], FP16, tag="vT")
                            if (k0 // 4) % 2 == 0:
                                nc.scalar.activation(
                                    out=vT4[:, 0:nk * P], in_=v4[:, 0:nk * P],
                                    func=AF.Relu)
                            else:
                                nc.vector.tensor_scalar_max(
                                    out=vT4[:, 0:nk * P], in0=v4[:, 0:nk * P],
                                    scalar1=0.0)
                            for j in range(nk):
                                k = k0 + j
                                sl = vT4[:, j * P:(j + 1) * P]
                                wk = w_t[:, k:k + 1]
                                if k == 0:
                                    nc.vector.tensor_scalar_mul(
                                        out=accA[:], in0=sl, scalar1=wk)
                                elif k == 1:
                                    nc.vector.tensor_scalar_mul(
                                        out=accB[:], in0=sl, scalar1=wk)
                                elif k % 2 == 0:
                                    nc.vector.scalar_tensor_tensor(
                                        out=accA[:], in0=sl, scalar=wk,
                                        op0=ALU.mult, in1=accA[:], op1=ALU.add)
                                else:
                                    nc.vector.scalar_tensor_tensor(
                                        out=accB[:], in0=sl, scalar=wk,
                                        op0=ALU.mult, in1=accB[:], op1=ALU.add)
                        acc = dwp.tile([P, P], FP16, tag="acc")
                        if KT == 1:
                            nc.vector.tensor_copy(out=acc[:], in_=accA[:])
                        else:
                            nc.vector.tensor_tensor(
                                out=acc[:], in0=accA[:], in1=accB[:],
                                op=ALU.add)

                        accT_ps = dt1ps.tile([P, P], FP16, tag="accT")
                        nc.tensor.transpose(out=accT_ps[:], in_=acc[:],
                                            identity=ident16[:])
                        accT = dwp.tile([P, P], FP16, tag="accTs")
                        nc.scalar.copy(out=accT[:], in_=accT_ps[:])
                        o_ps = dops.tile([P, P], F32, tag="o")
                        nc.tensor.matmul(out=o_ps[:], lhsT=wo_sb[:], rhs=accT[:],
                                         start=True, stop=False)
                        nc.tensor.matmul(out=o_ps[:], lhsT=wpw_sb[:], rhs=a4T[:],
                                         start=False, stop=True)
                        oT = dwp.tile([P, P], F32, tag="oT")
                        nc.scalar.activation(
                            out=oT[:], in_=o_ps[:], func=AF.Relu,
                            bias=obeta_sb[:, 0:1])
                        res = dwp.tile([P, P], F32, tag="res")
                        nc.vector.tensor_tensor(out=res[:], in0=oT[:],
                                                in1=xo_t[:], op=ALU.add)
                        nc.sync.dma_start(out=outT[:, t * P:(t + 1) * P],
                                          in_=res[:])
                cstk.close()

    nc.finalize()
    return nc


def _prep(inputs):
    x = np.asarray(inputs["x"], np.float32)
    coords = np.asarray(inputs["coords"], np.float32)
    W_q = np.asarray(inputs["W_q"], np.float32)
    q_gamma = np.asarray(inputs["q_gamma"], np.float32)
    q_beta = np.asarray(inputs["q_beta"], np.float32)
    W_v = np.asarray(inputs["W_v"], np.float32)
    v_gamma = np.asarray(inputs["v_gamma"], np.float32)
    v_beta = np.asarray(inputs["v_beta"], np.float32)
    codebook = np.asarray(inputs["codebook"], np.float32)
    W_choice = np.asarray(inputs["W_choice"], np.float32)
    b_choice = np.asarray(inputs["b_choice"], np.float32)
    W_pos = np.asarray(inputs["W_pos"], np.float32)
    b_pos = np.asarray(inputs["b_pos"], np.float32)
    W_out = np.asarray(inputs["W_out"], np.float32)
    out_gamma = np.asarray(inputs["out_gamma"], np.float32)
    out_beta = np.asarray(inputs["out_beta"], np.float32)
    nbr_idx = np.asarray(inputs["nbr_idx"], np.int32)
    nbr_mask = np.asarray(inputs["nbr_mask"], np.int32)

    n = x.shape[0]
    assert n == N

    # ---- valid-degree sort (per core shard) -> global relabeling ----
    mask_pad = np.zeros((K, NTOT), bool)
    mask_pad[:, :n] = nbr_mask > 0
    deg = mask_pad.sum(0)
    orders = []
    degs_sorted = np.empty((NCORE, NSH), np.int64)
    for r in range(NCORE):
        sl = slice(r * NSH, (r + 1) * NSH)
        o = np.argsort(-deg[sl], kind="stable")
        orders.append(o)
        degs_sorted[r] = deg[sl][o]
    kts = tuple(int(max(1, degs_sorted[:, t * P:(t + 1) * P].max()))
                for t in range(TO))
    SUMK = sum(kts)
    perm_full = np.concatenate([r * NSH + orders[r] for r in range(NCORE)])
    inv = np.empty(NTOT, np.int64)
    inv[perm_full] = np.arange(NTOT)

    # ---- permuted global tables (new-id order) ----
    xp = np.zeros((NTOT, P), np.float32)
    xp[:n] = x
    xp2 = xp[perm_full]
    x16g = xp2.astype(np.float16)
    cp = np.zeros((NTOT, 3), np.float32)
    cp[:n] = coords
    c4g = np.ones((NTOT, 4), np.float32)
    c4g[:, :3] = cp[perm_full]

    # ---- weight folds ----
    cb2 = float(np.dot(codebook, codebook))
    scb = np.sqrt(cb2).astype(np.float32)
    wcp = codebook[:, None] * W_choice
    wcc = scb * wcp.reshape(VEC, P // VEC, P).sum(1)
    bch = (scb * b_choice)[None, :]
    use_bch = bool(np.any(b_choice != 0))
    wq_flat = np.ascontiguousarray(
        W_q.transpose(1, 0, 2).reshape(P, K * VEC)).astype(np.float16)
    wv16 = (W_v * v_gamma[None, :]).astype(np.float16)
    use_vb = bool(np.any(v_beta != 0))
    wo = W_out * out_gamma[None, :]
    wo16 = wo.astype(np.float16)
    woB = wo.reshape(VEC, P // VEC, P).sum(1)          # [16, 128]
    wpos4 = np.concatenate([W_pos, b_pos[None, :]], axis=0)  # [4, 16]
    wpw16 = (wpos4 @ woB).astype(np.float16)           # [4, 128]
    rmio = np.tile(np.arange(32, dtype=np.float16)[None, :], (P, 1))

    # ---- per-slot neighbor ids (new ids, valid-first compaction) ----
    idx_new = np.full((K, NTOT), Z, np.int32)
    idx_new[:, :n] = np.where(nbr_mask > 0, inv[nbr_idx], Z).astype(np.int32)
    bias_pad = np.full((K, NTOT), np.float32(NEG), np.float32)
    bias_pad[:, :n] = np.where(nbr_mask > 0, 0.0, NEG).astype(np.float32)
    idx_km = idx_new[:, perm_full]          # k-major (original offsets)
    korder = np.argsort(~mask_pad, axis=0, kind="stable")   # valid ks first
    idx_new = np.take_along_axis(idx_new, korder, axis=0)
    bias_pad = np.take_along_axis(bias_pad, korder, axis=0)
    # permute slot-grid columns to sorted point order
    idx_new = idx_new[:, perm_full]
    bias_pad = bias_pad[:, perm_full]

    shared = dict(w_q=wq_flat, wcc=wcc, bch=bch, wv=wv16, wo=wo16,
                  wpw=wpw16, qg=q_gamma[:, None], qb=q_beta[:, None],
                  vbeta=v_beta[:, None], obeta=out_beta[:, None], rmio=rmio)
    if use_vb:
        shared["vbr"] = v_beta[None, :].astype(np.float16)

    prow = np.arange(P, dtype=np.int64)
    in_maps = []
    for r in range(NCORE):
        sl = slice(r * NSH, (r + 1) * NSH)
        slots = idx_new[:, sl]      # [K, NSH] new ids (compacted)
        biasr = bias_pad[:, sl]     # [K, NSH]
        # k-major edge-expanded x for phase A: [128, TO*K*128]
        ja = idx_km[:, sl]          # [K, NSH]
        jlA = ja.reshape(K, TO, P).transpose(1, 0, 2).ravel()  # (t, k, p)
        xeA_r = np.ascontiguousarray(x16g[jlA].T)

        jl_parts = []
        aux_parts = []
        ilo_parts = []
        ihi_parts = []
        code_parts = []
        for t in range(TO):
            KT = kts[t]
            s_tk = slots[:KT, t * P:(t + 1) * P]      # [KT, 128] (k, p)
            b_tk = biasr[:KT, t * P:(t + 1) * P]
            jl_parts.append(s_tk.ravel())             # (k, p) order
            # aux: [128, KT, 5] -> per-partition (k-major) c4 + bias
            a = np.empty((P, KT, 5), np.float32)
            a[:, :, :4] = c4g[s_tk.T]                 # [128, KT, 4]
            a[:, :, 4] = b_tk.T
            aux_parts.append(a.reshape(P, KT * 5))
            # ce lookup tables
            nn = s_tk.T.astype(np.int64)              # [128, KT]
            valid = b_tk.T == 0.0
            fpn = (nn % P) * COLS + nn // P
            slab = fpn // HALFV
            w_in = fpn % HALFV
            ent = w_in // 2 + 1
            m = fpn % 2
            ilo = np.where(slab == 0, ent, 0).astype(np.int16)
            ihi = np.where(slab == 1, ent, 0).astype(np.int16)
            code = np.where(valid, (prow[:, None] % 16) * 2 + m,
                            -1).astype(np.float16)
            ilo_parts.append(np.concatenate([ilo, ihi], axis=1))
            code_parts.append(code)

        jl = np.concatenate(jl_parts)                 # [SUMK*128]
        xeT_r = np.ascontiguousarray(x16g[jl].T)      # [128, SUMK*128]
        aux_r = np.ascontiguousarray(np.concatenate(aux_parts, axis=1))
        pki_r = np.ascontiguousarray(np.concatenate(ilo_parts, axis=1))
        pkc_r = np.ascontiguousarray(np.concatenate(code_parts, axis=1))

        m = dict(shared)
        m["xeA"] = xeA_r
        m["xeT"] = xeT_r
        m["aux"] = aux_r
        m["pki"] = pki_r
        m["pkc"] = pkc_r
        m["xT_own"] = np.ascontiguousarray(xp2[sl].T)
        in_maps.append(m)
    return in_maps, kts, orders, use_bch, use_vb


def prepare(inputs):
    in_maps, kts, orders, use_bch, use_vb = _prep(inputs)
    key = (kts, use_bch, use_vb)
    if _CACHE.get("key") != key:
        _CACHE["nc"] = _build_nc(kts, use_bch, use_vb)
        _CACHE["key"] = key
    return _CACHE["nc"], in_maps, orders


def assemble(results, orders):
    out = np.empty((NCORE * NSH, P), np.float32)
    for r in range(NCORE):
        out[r * NSH + orders[r]] = results[r]["outT"].T
    return np.ascontiguousarray(out[:N])


def kernel(**inputs):
    nc, in_maps, orders = prepare(inputs)
    res = run_bass_kernel_spmd(nc, in_maps, list(range(NCORE)))
    return assemble(res.results, orders)


if __name__ == "__main__":
    rng = np.random.default_rng(0)
    ins = dict(
        x=rng.standard_normal((N, P)).astype(np.float32),
        coords=(rng.random((N, 3)) * 100).astype(np.float32),
        W_q=rng.standard_normal((K, P, VEC)).astype(np.float32) * (P * K) ** -0.5,
        q_gamma=np.ones(VEC, np.float32), q_beta=np.zeros(VEC, np.float32),
        W_v=rng.standard_normal((P, P)).astype(np.float32) * P ** -0.5,
        v_gamma=np.ones(P, np.float32), v_beta=np.zeros(P, np.float32),
        codebook=rng.standard_normal(P).astype(np.float32) * 0.1,
        W_choice=rng.standard_normal((P, P)).astype(np.float32) * P ** -0.5,
        b_choice=np.zeros(P, np.float32),
        W_pos=rng.standard_normal((3, VEC)).astype(np.float32) * 3 ** -0.5,
        b_pos=np.zeros(VEC, np.float32),
        W_out=rng.standard_normal((P, P)).astype(np.float32) * P ** -0.5,
        out_gamma=np.ones(P, np.float32), out_beta=np.zeros(P, np.float32),
        nbr_idx=rng.integers(0, N, (K, N)).astype(np.int32),
        nbr_mask=rng.integers(0, 2, (K, N)).astype(np.int32),
    )
    out = kernel(**ins)
    print("kernel output", out.shape, out.dtype)
